# revision 4
# baseline (speedup 1.0000x reference)
"""GNN message-passing + pooling kernel for 8 Trainium2 NeuronCores.

Strategy (per the sharding hint):
  - Host: sort edges by dst, partition the 50k nodes into 8 contiguous
    ranges of 6250; each core gets the edges targeting its node range
    (disjoint scatter -> no cross-core reduction needed).
  - Host gathers x[dst], x[src], edge_attr into a transposed bf16
    [320, E_pad] tensor per core (edges grouped into 481-node scatter
    windows, padded to a uniform chunk count so the device program is
    identical across cores).
  - Device (per core): 4-layer message MLP in transposed-activation
    layout processed in 2048-edge blocks (4x512 supertiles).  Each
    weight chunk is kept stationary on the PE array for 4 consecutive
    matmuls (amortizes LDWEIGHTS, which otherwise serializes ~100ns per
    matmul).  Layer 4 is computed weight-stationary into a feature-major
    [msg_dim, edges] PSUM tile, bias fused into the PSUM->SBUF copy on
    the scalar engine, then DMA-transposed (xbar) into edge-major
    [128, 128] chunks for the scatter.  Scatter-add via one-hot matmuls
    (one-hot built on DVE with iota + is_equal against per-edge local
    dst), deferred by one block so the transposes are off the critical
    path.  Node MLP over the core's 6250 nodes with the same blocked
    structure, per-graph sum-pooling accumulated in a single PSUM bank.
    Output: [32, 128] partial per-graph sums.
  - Host: sum the 8 partials, divide by per-graph node counts, apply the
    final [128, 16] linear.
"""

import sys

if "/opt/trn_rl_repo" not in sys.path:
    sys.path.insert(0, "/opt/trn_rl_repo")

import numpy as np
import ml_dtypes

BF16 = ml_dtypes.bfloat16

# Problem dims
N_NODES = 50000
N_EDGES = 800000
NF = 128          # node feature dim
EF = 64           # edge feature dim
MSGD = 128        # message dim
HID = 300         # MLP hidden
G = 32            # graphs
NCORES = 8

# Tiling config
NPC = N_NODES // NCORES   # 6250 nodes per core
NW = 481                  # nodes per scatter window
W = 13                    # windows per core (13*481 = 6253 >= 6250)
ST = 512                  # edge supertile (free dim per matmul)
BST = 4                   # supertiles per block (weight-stationary reuse)
NP2 = 6656                # padded nodes per core for node MLP (13*512)
NT = NP2 // ST            # node supertiles
NCHK = NP2 // 128         # node chunks for pooling

TRACE = False             # set True from test harness to profile core 0
LAST_EXEC_NS = None

_BUILD_CACHE = {}


def _chunks(total, step=128):
    return [(o, min(step, total - o)) for o in range(0, total, step)]


def _build_nc(C):
    """Build the (single) SPMD Bass program. C = 128-edge chunks per window
    (multiple of 16 so each window is a whole number of 2048-edge blocks)."""
    import concourse.bacc as bacc
    import concourse.tile as tile
    from concourse import mybir
    from contextlib import ExitStack

    f32 = mybir.dt.float32
    bf16 = mybir.dt.bfloat16
    AF = mybir.ActivationFunctionType
    OP = mybir.AluOpType

    E_pad = W * C * 128
    NCHUNKS = W * C
    NBLK = NCHUNKS // 16      # 2048-edge blocks

    nc = bacc.Bacc("TRN2", target_bir_lowering=False, debug=False,
                   num_devices=NCORES)

    # --- DRAM I/O ---
    d_msg_inT = nc.dram_tensor("msg_inT", [2 * NF + EF, E_pad], bf16,
                               kind="ExternalInput")
    d_dstloc = nc.dram_tensor("dstloc", [128, NCHUNKS], f32,
                              kind="ExternalInput")
    d_xT = nc.dram_tensor("xT", [NF, NP2], bf16, kind="ExternalInput")
    d_pmat = nc.dram_tensor("pmat", [128, NCHK * G], bf16,
                            kind="ExternalInput")
    d_mW = [nc.dram_tensor(f"mW{i}", s, bf16, kind="ExternalInput")
            for i, s in enumerate([[2 * NF + EF, HID], [HID, HID], [HID, HID],
                                   [HID, MSGD]], start=1)]
    d_mb = [nc.dram_tensor(f"mb{i}", [HID, 1], f32, kind="ExternalInput")
            for i in range(1, 4)]
    d_mb4c = nc.dram_tensor("mb4c", [MSGD, 1], f32, kind="ExternalInput")
    d_nW = [nc.dram_tensor(f"nW{i}", s, bf16, kind="ExternalInput")
            for i, s in enumerate([[NF + MSGD, HID], [HID, HID], [HID, HID],
                                   [HID, NF]], start=1)]
    d_nb = [nc.dram_tensor(f"nb{i}", [HID, 1], f32, kind="ExternalInput")
            for i in range(1, 4)]
    d_nb4r = nc.dram_tensor("nb4r", [1, NF], bf16, kind="ExternalInput")
    d_out = nc.dram_tensor("partial", [G, NF], f32, kind="ExternalOutput")

    HCH = _chunks(HID)          # [(0,128),(128,128),(256,44)]
    KIN = _chunks(2 * NF + EF)  # [(0,128),(128,128),(256,64)]

    with tile.TileContext(nc) as tc, ExitStack() as ctx:
        wpool = ctx.enter_context(tc.tile_pool(name="w", bufs=1))
        apool = ctx.enter_context(tc.tile_pool(name="agg", bufs=1))
        inpool = ctx.enter_context(tc.tile_pool(name="in", bufs=2))
        hpool = ctx.enter_context(tc.tile_pool(name="h", bufs=2))
        mtpool = ctx.enter_context(tc.tile_pool(name="mt", bufs=2))
        mpool = ctx.enter_context(tc.tile_pool(name="m", bufs=4))
        spool = ctx.enter_context(tc.tile_pool(name="s", bufs=8))
        mm_psum = ctx.enter_context(
            tc.tile_pool(name="mmp", bufs=7, space="PSUM"))
        acc_psum = ctx.enter_context(
            tc.tile_pool(name="accp", bufs=1, space="PSUM"))

        def load_w(dram, K, N, dt, name):
            tiles = []
            for i, (k0, kk) in enumerate(_chunks(K)):
                t = wpool.tile([kk, N], dt, tag=f"{name}{i}")
                nc.sync.dma_start(t[:, :], dram[k0:k0 + kk, :])
                tiles.append(t)
            return tiles

        mW = [load_w(d_mW[0], 2 * NF + EF, HID, bf16, "mW1"),
              load_w(d_mW[1], HID, HID, bf16, "mW2"),
              load_w(d_mW[2], HID, HID, bf16, "mW3"),
              load_w(d_mW[3], HID, MSGD, bf16, "mW4")]
        mb = [load_w(d_mb[i], HID, 1, f32, f"mb{i + 1}") for i in range(3)]
        nW = [load_w(d_nW[0], NF + MSGD, HID, bf16, "nW1"),
              load_w(d_nW[1], HID, HID, bf16, "nW2"),
              load_w(d_nW[2], HID, HID, bf16, "nW3"),
              load_w(d_nW[3], HID, NF, bf16, "nW4")]
        nb = [load_w(d_nb[i], HID, 1, f32, f"nb{i + 1}") for i in range(3)]
        mb4c = wpool.tile([MSGD, 1], f32, tag="mb4c")
        nc.sync.dma_start(mb4c[:, :], d_mb4c[:, :])
        nb4r = wpool.tile([1, NF], bf16, tag="nb4r")
        nc.sync.dma_start(nb4r[:, :], d_nb4r[:, :])

        dstloc = wpool.tile([128, NCHUNKS], f32, tag="dstloc")
        nc.sync.dma_start(dstloc[:, :], d_dstloc[:, :])
        xT = wpool.tile([NF, NP2], bf16, tag="xT")
        nc.sync.dma_start(xT[:, :], d_xT[:, :])
        pmat = wpool.tile([128, NCHK * G], bf16, tag="pmat")
        nc.sync.dma_start(pmat[:, :], d_pmat[:, :])

        iota = wpool.tile([128, NW], f32, tag="iota")
        nc.gpsimd.iota(iota[:, :], pattern=[[1, NW]], base=0,
                       channel_multiplier=0,
                       allow_small_or_imprecise_dtypes=True)
        ones1 = wpool.tile([1, 128], bf16, tag="ones1")
        nc.gpsimd.memset(ones1[:, :], 1.0)

        aggrT = apool.tile([NF, NP2], bf16, tag="aggrT")
        # scatter windows cover cols [0, W*NW); zero the tail
        nc.gpsimd.memset(aggrT[:, W * NW:NP2], 0.0)

        def mlp_front(ksrc_fn, kch, weights, biases, gs):
            """Layers 1-3, weight-stationary over gs supertiles.

            ksrc_fn(layer, prev) -> list of (get_rhs(g) -> AP, kk).
            Returns the layer-3 h tiles."""
            prev = None
            for layer in range(3):
                ksrc = ksrc_fn(layer, prev)
                cur = []
                for m, (m0, mm) in enumerate(HCH):
                    pss = []
                    for g in range(gs):
                        p = mm_psum.tile([128, ST], f32, tag="mmp")
                        pss.append(p)
                    for k, (get_rhs, kk) in enumerate(ksrc):
                        lhs = weights[layer][k][:, m0:m0 + mm]
                        for g in range(gs):
                            nc.tensor.matmul(pss[g][:mm, :], lhs, get_rhs(g),
                                             start=(k == 0),
                                             stop=(k == len(ksrc) - 1))
                    ht = hpool.tile([HCH[m][1], BST * ST], bf16,
                                    tag=f"h{layer}_{m}")
                    for g in range(gs):
                        if layer == 1:
                            nc.vector.tensor_scalar(
                                ht[:mm, g * ST:(g + 1) * ST], pss[g][:mm, :],
                                biases[layer][m][:mm, :], 0.0,
                                op0=OP.add, op1=OP.max)
                        else:
                            nc.scalar.activation(
                                ht[:mm, g * ST:(g + 1) * ST], pss[g][:mm, :],
                                AF.Relu, bias=biases[layer][m][:mm, :])
                    cur.append(ht)
                prev = cur
            return prev

        # ---- edge phase ----
        acc_state = {}

        def emit_scatter(blk, msgts):
            for i, mt in enumerate(msgts):
                cidx = blk * 16 + i
                w = cidx // C
                cw = cidx % C
                if cw == 0:
                    at = acc_psum.tile([128, NW], f32, tag="acc")
                    acc_state["t"] = at
                st = spool.tile([128, NW], bf16, tag="st")
                nc.vector.tensor_scalar(
                    st[:, :], iota[:, :], dstloc[:, cidx:cidx + 1], None,
                    op0=OP.is_equal)
                nc.tensor.matmul(acc_state["t"][:, :], mt[:, :], st[:, :],
                                 start=(cw == 0), stop=(cw == C - 1),
                                 skip_group_check=True)
                if cw == C - 1:
                    nc.vector.tensor_copy(
                        aggrT[:, w * NW:(w + 1) * NW], acc_state["t"][:, :])

        prev_msgts = None
        prev_blk = None
        for blk in range(NBLK):
            base = blk * (BST * ST)
            in_t = []
            for i, (k0, kk) in enumerate(KIN):
                t = inpool.tile([kk, BST * ST], bf16, tag=f"in{i}")
                nc.sync.dma_start(t[:, :],
                                  d_msg_inT[k0:k0 + kk, base:base + BST * ST])
                in_t.append(t)

            def eksrc(layer, prev):
                if layer == 0:
                    return [
                        ((lambda g, t=t, kk=kk: t[:kk, g * ST:(g + 1) * ST]),
                         kk) for t, (k0, kk) in zip(in_t, KIN)]
                return [
                    ((lambda g, h=prev[i], kk=kk: h[:kk, g * ST:(g + 1) * ST]),
                     kk) for i, (k0, kk) in enumerate(HCH)]

            h3 = mlp_front(eksrc, KIN, mW, mb, BST)

            # L4 weight-stationary: out [MSGD, ST] feature-major
            psl4 = []
            for g in range(BST):
                p4 = mm_psum.tile([128, ST], f32, tag="mmp")
                psl4.append(p4)
            for k, (k0, kk) in enumerate(HCH):
                for g in range(BST):
                    nc.tensor.matmul(psl4[g][:, :], mW[3][k][:kk, :],
                                     h3[k][:kk, g * ST:(g + 1) * ST],
                                     start=(k == 0), stop=(k == 2))
            msgts = []
            for g in range(BST):
                mT = mtpool.tile([128, ST], bf16, tag=f"msgT{g}")
                nc.scalar.activation(mT[:, :], psl4[g][:, :], AF.Identity,
                                     bias=mb4c[:, :])
                for e in range(4):
                    i = g * 4 + e
                    mt = mtpool.tile([128, 128], bf16, tag=f"msgt{i}")
                    nc.scalar.dma_start_transpose(
                        mt[:, :], mT[:, e * 128:(e + 1) * 128])
                    msgts.append(mt)

            if prev_msgts is not None:
                emit_scatter(prev_blk, prev_msgts)
            prev_msgts, prev_blk = msgts, blk
        emit_scatter(prev_blk, prev_msgts)

        # ---- node phase ----
        pool_acc = acc_psum.tile([128, NW], f32, tag="acc")
        for t0 in range(0, NT, BST):
            gs = min(BST, NT - t0)

            def nksrc(layer, prev, t0=t0):
                if layer == 0:
                    return [
                        ((lambda g, s=xT, t0=t0:
                          s[:, (t0 + g) * ST:(t0 + g + 1) * ST]), NF),
                        ((lambda g, s=aggrT, t0=t0:
                          s[:, (t0 + g) * ST:(t0 + g + 1) * ST]), MSGD)]
                return [
                    ((lambda g, h=prev[i], kk=kk: h[:kk, g * ST:(g + 1) * ST]),
                     kk) for i, (k0, kk) in enumerate(HCH)]

            h3n = mlp_front(nksrc, None, nW, nb, gs)

            for g in range(gs):
                for e in range(4):
                    tch = (t0 + g) * 4 + e
                    ps = mm_psum.tile([128, ST], f32, tag="mmp")
                    for k, (k0, kk) in enumerate(HCH):
                        nc.tensor.matmul(
                            ps[:, :NF],
                            h3n[k][:kk, g * ST + e * 128:g * ST + (e + 1) * 128],
                            nW[3][k][:kk, :], start=(k == 0), stop=False)
                    nc.tensor.matmul(ps[:, :NF], ones1[:1, :], nb4r[:1, :],
                                     start=False, stop=True)
                    no = mpool.tile([128, NF], bf16, tag="no")
                    nc.scalar.activation(no[:, :], ps[:, :NF], AF.Copy)
                    nc.tensor.matmul(pool_acc[:G, :NF],
                                     pmat[:, tch * G:(tch + 1) * G], no[:, :],
                                     start=(tch == 0), stop=(tch == NCHK - 1),
                                     skip_group_check=True)

        pooled = apool.tile([G, NF], f32, tag="pooled")
        nc.vector.tensor_copy(pooled[:, :], pool_acc[:G, :NF])
        nc.sync.dma_start(d_out[:, :], pooled[:, :])

    nc.compile()
    return nc


def _prep_inputs(x, edge_index, edge_attr, batch, weights, C):
    """Host-side shard/gather/pad. Returns per-core in_maps."""
    E_pad = W * C * 128
    src = np.asarray(edge_index[0], np.int64)
    dst = np.asarray(edge_index[1], np.int64)

    order = np.argsort(dst, kind="stable")
    dsts = dst[order]
    srcs = src[order]

    xT = np.ascontiguousarray(np.asarray(x, np.float32).astype(BF16).T)
    eaT = np.ascontiguousarray(np.asarray(edge_attr, np.float32).astype(BF16).T)
    batch = np.asarray(batch, np.int64)

    bounds = np.searchsorted(dsts, np.arange(0, N_NODES + 1, NPC))

    wcommon = {}
    for i in range(1, 5):
        wcommon[f"mW{i}"] = np.ascontiguousarray(
            weights[f"mW{i}"].astype(BF16))
        wcommon[f"nW{i}"] = np.ascontiguousarray(
            weights[f"nW{i}"].astype(BF16))
    for i in range(1, 4):
        wcommon[f"mb{i}"] = np.ascontiguousarray(
            weights[f"mb{i}"].reshape(HID, 1).astype(np.float32))
        wcommon[f"nb{i}"] = np.ascontiguousarray(
            weights[f"nb{i}"].reshape(HID, 1).astype(np.float32))
    wcommon["mb4c"] = np.ascontiguousarray(
        weights["mb4"].reshape(MSGD, 1).astype(np.float32))
    wcommon["nb4r"] = np.ascontiguousarray(
        weights["nb4"].reshape(1, NF).astype(BF16))

    garange = np.arange(G)
    in_maps = []
    for k in range(NCORES):
        sl = slice(int(bounds[k]), int(bounds[k + 1]))
        eidx = order[sl]
        dloc = dsts[sl] - k * NPC
        srck = srcs[sl]
        win = dloc // NW
        cnt = np.bincount(win, minlength=W)

        starts = np.repeat(np.arange(W) * C * 128, cnt)
        within = np.arange(len(dloc)) - np.repeat(np.cumsum(cnt) - cnt, cnt)
        pos = starts + within

        msg_inT = np.zeros((2 * NF + EF, E_pad), BF16)
        msg_inT[0:NF, pos] = xT[:, k * NPC + dloc]
        msg_inT[NF:2 * NF, pos] = xT[:, srck]
        msg_inT[2 * NF:, pos] = eaT[:, eidx]

        dl = np.full(E_pad, -1.0, np.float32)
        dl[pos] = (dloc - win * NW).astype(np.float32)
        dstloc = np.ascontiguousarray(dl.reshape(E_pad // 128, 128).T)

        xTn = np.zeros((NF, NP2), BF16)
        xTn[:, :NPC] = xT[:, k * NPC:(k + 1) * NPC]

        bl = np.full(NP2, -1, np.int64)
        bl[:NPC] = batch[k * NPC:(k + 1) * NPC]
        P = (bl[:, None] == garange[None, :]).astype(BF16)
        pmat = np.ascontiguousarray(
            P.reshape(NCHK, 128, G).transpose(1, 0, 2).reshape(128, NCHK * G))

        in_map = dict(wcommon)
        in_map.update(msg_inT=msg_inT, dstloc=dstloc, xT=xTn, pmat=pmat)
        in_maps.append(in_map)
    return in_maps


def kernel(**inputs):
    global LAST_EXEC_NS
    from concourse.bass_utils import run_bass_kernel_spmd

    x = np.asarray(inputs["x"], np.float32)
    edge_index = np.asarray(inputs["edge_index"])
    edge_attr = np.asarray(inputs["edge_attr"], np.float32)
    batch = np.asarray(inputs["batch"])

    # chunk count per window from the actual data (uniform across cores)
    dst = np.asarray(edge_index[1], np.int64)
    dloc_all = dst % NPC
    core_all = dst // NPC
    win_all = dloc_all // NW
    cnt = np.bincount(core_all * W + win_all, minlength=NCORES * W)
    C = int(np.ceil(cnt.max() / 128.0))
    C = max(16, int(np.ceil(C / 16.0)) * 16)

    key = C
    if key not in _BUILD_CACHE:
        _BUILD_CACHE[key] = _build_nc(C)
    nc = _BUILD_CACHE[key]

    in_maps = _prep_inputs(x, edge_index, edge_attr, batch, inputs, C)

    res = run_bass_kernel_spmd(nc, in_maps, list(range(NCORES)), trace=TRACE)
    LAST_EXEC_NS = res.exec_time_ns

    total = np.zeros((G, NF), np.float64)
    for r in res.results:
        total += np.asarray(r["partial"], np.float64)

    counts = np.bincount(np.asarray(batch, np.int64), minlength=G)
    pooled = (total / np.maximum(counts, 1)[:, None]).astype(np.float32)
    out = pooled @ np.asarray(inputs["linW"], np.float32) + np.asarray(
        inputs["linb"], np.float32)
    return out.astype(np.float32)


# revision 8
# speedup vs baseline: 1.5264x; 1.5264x over previous
"""GNN message-passing + pooling kernel for 8 Trainium2 NeuronCores.

Strategy (per the sharding hint):
  - Host: sort edges by dst, partition the 50k nodes into 8 contiguous
    ranges of 6250; each core gets the edges targeting its node range
    (disjoint scatter -> no cross-core reduction needed).
  - Host gathers x[dst], x[src], edge_attr into a transposed bf16
    [320, E_pad] tensor per core (edges grouped into 481-node scatter
    windows, padded to a uniform chunk count so the device program is
    identical across cores).
  - Device (per core): 4-layer message MLP in transposed-activation
    layout processed in 2048-edge blocks (4x512 supertiles).  Each
    weight chunk is kept stationary on the PE array for 4 consecutive
    matmuls (amortizes LDWEIGHTS, which otherwise serializes ~100ns per
    matmul).  Layer 4 is computed weight-stationary into a feature-major
    [msg_dim, edges] PSUM tile, bias fused into the PSUM->SBUF copy on
    the scalar engine, then DMA-transposed (xbar) into edge-major
    [128, 128] chunks for the scatter.  Scatter-add via one-hot matmuls
    (one-hot built on DVE with iota + is_equal against per-edge local
    dst), deferred by one block so the transposes are off the critical
    path.  Node MLP over the core's 6250 nodes with the same blocked
    structure, per-graph sum-pooling accumulated in a single PSUM bank.
    Output: [32, 128] partial per-graph sums.
  - Host: sum the 8 partials, divide by per-graph node counts, apply the
    final [128, 16] linear.
"""

import sys

if "/opt/trn_rl_repo" not in sys.path:
    sys.path.insert(0, "/opt/trn_rl_repo")

import numpy as np
import ml_dtypes

BF16 = ml_dtypes.bfloat16

# Problem dims
N_NODES = 50000
N_EDGES = 800000
NF = 128          # node feature dim
EF = 64           # edge feature dim
MSGD = 128        # message dim
HID = 300         # MLP hidden
G = 32            # graphs
NCORES = 8

# Tiling config
NPC = N_NODES // NCORES   # 6250 nodes per core
NW = 481                  # nodes per scatter window
W = 13                    # windows per core (13*481 = 6253 >= 6250)
ST = 512                  # edge supertile (free dim per matmul)
BST = 4                   # supertiles per block (weight-stationary reuse)
NP2 = 6656                # padded nodes per core for node MLP (13*512)
NT = NP2 // ST            # node supertiles
NCHK = NP2 // 128         # node chunks for pooling

TRACE = False             # set True from test harness to profile core 0
LAST_EXEC_NS = None

_BUILD_CACHE = {}


def _chunks(total, step=128):
    return [(o, min(step, total - o)) for o in range(0, total, step)]


def _build_nc(C):
    """Build the (single) SPMD Bass program. C = 128-edge chunks per window
    (multiple of 16 so each window is a whole number of 2048-edge blocks)."""
    import concourse.bacc as bacc
    import concourse.tile as tile
    from concourse import mybir
    from contextlib import ExitStack

    f32 = mybir.dt.float32
    bf16 = mybir.dt.bfloat16
    AF = mybir.ActivationFunctionType
    OP = mybir.AluOpType

    E_pad = W * C * 128
    NCHUNKS = W * C
    NBLK = NCHUNKS // 16      # 2048-edge blocks

    nc = bacc.Bacc("TRN2", target_bir_lowering=False, debug=False,
                   num_devices=NCORES)

    # --- DRAM I/O ---
    d_msg_inT = nc.dram_tensor("msg_inT", [2 * NF + EF, E_pad], bf16,
                               kind="ExternalInput")
    d_dstloc = nc.dram_tensor("dstloc", [128, NCHUNKS], f32,
                              kind="ExternalInput")
    d_xT = nc.dram_tensor("xT", [NF, NP2], bf16, kind="ExternalInput")
    d_pmat = nc.dram_tensor("pmat", [128, NCHK * G], bf16,
                            kind="ExternalInput")
    d_mW = [nc.dram_tensor(f"mW{i}", s, bf16, kind="ExternalInput")
            for i, s in enumerate([[2 * NF + EF, HID], [HID, HID], [HID, HID],
                                   [HID, MSGD]], start=1)]
    d_mb = [nc.dram_tensor(f"mb{i}", [HID, 1], f32, kind="ExternalInput")
            for i in range(1, 4)]
    d_mb4c = nc.dram_tensor("mb4c", [MSGD, 1], f32, kind="ExternalInput")
    d_nW = [nc.dram_tensor(f"nW{i}", s, bf16, kind="ExternalInput")
            for i, s in enumerate([[NF + MSGD, HID], [HID, HID], [HID, HID],
                                   [HID, NF]], start=1)]
    d_nb = [nc.dram_tensor(f"nb{i}", [HID, 1], f32, kind="ExternalInput")
            for i in range(1, 4)]
    d_nb4r = nc.dram_tensor("nb4r", [1, NF], bf16, kind="ExternalInput")
    d_out = nc.dram_tensor("partial", [G, NF], f32, kind="ExternalOutput")

    HCH = _chunks(HID)          # [(0,128),(128,128),(256,44)]
    KIN = _chunks(2 * NF + EF)  # [(0,128),(128,128),(256,64)]

    with tile.TileContext(nc) as tc, ExitStack() as ctx:
        wpool = ctx.enter_context(tc.tile_pool(name="w", bufs=1))
        apool = ctx.enter_context(tc.tile_pool(name="agg", bufs=1))
        inpool = ctx.enter_context(tc.tile_pool(name="in", bufs=3))
        hpool = ctx.enter_context(tc.tile_pool(name="h", bufs=2))
        mtpool = ctx.enter_context(tc.tile_pool(name="mt", bufs=2))
        mpool = ctx.enter_context(tc.tile_pool(name="m", bufs=4))
        spool = ctx.enter_context(tc.tile_pool(name="s", bufs=8))
        mm_psum = ctx.enter_context(
            tc.tile_pool(name="mmp", bufs=7, space="PSUM"))
        acc_psum = ctx.enter_context(
            tc.tile_pool(name="accp", bufs=1, space="PSUM"))

        def load_w(dram, K, N, dt, name):
            tiles = []
            for i, (k0, kk) in enumerate(_chunks(K)):
                t = wpool.tile([kk, N], dt, tag=f"{name}{i}")
                nc.sync.dma_start(t[:, :], dram[k0:k0 + kk, :])
                tiles.append(t)
            return tiles

        mW = [load_w(d_mW[0], 2 * NF + EF, HID, bf16, "mW1"),
              load_w(d_mW[1], HID, HID, bf16, "mW2"),
              load_w(d_mW[2], HID, HID, bf16, "mW3"),
              load_w(d_mW[3], HID, MSGD, bf16, "mW4")]
        mb = [load_w(d_mb[i], HID, 1, f32, f"mb{i + 1}") for i in range(3)]
        nW = [load_w(d_nW[0], NF + MSGD, HID, bf16, "nW1"),
              load_w(d_nW[1], HID, HID, bf16, "nW2"),
              load_w(d_nW[2], HID, HID, bf16, "nW3"),
              load_w(d_nW[3], HID, NF, bf16, "nW4")]
        nb = [load_w(d_nb[i], HID, 1, f32, f"nb{i + 1}") for i in range(3)]
        mb4c = wpool.tile([MSGD, 1], f32, tag="mb4c")
        nc.sync.dma_start(mb4c[:, :], d_mb4c[:, :])
        nb4r = wpool.tile([1, NF], bf16, tag="nb4r")
        nc.sync.dma_start(nb4r[:, :], d_nb4r[:, :])

        dstloc = wpool.tile([128, NCHUNKS], f32, tag="dstloc")
        nc.sync.dma_start(dstloc[:, :], d_dstloc[:, :])
        xT = wpool.tile([NF, NP2], bf16, tag="xT")
        nc.sync.dma_start(xT[:, :], d_xT[:, :])
        pmat = wpool.tile([128, NCHK * G], bf16, tag="pmat")
        nc.sync.dma_start(pmat[:, :], d_pmat[:, :])

        iota = wpool.tile([128, NW], f32, tag="iota")
        nc.gpsimd.iota(iota[:, :], pattern=[[1, NW]], base=0,
                       channel_multiplier=0,
                       allow_small_or_imprecise_dtypes=True)
        ones1 = wpool.tile([1, 128], bf16, tag="ones1")
        nc.gpsimd.memset(ones1[:, :], 1.0)

        aggrT = apool.tile([NF, NP2], bf16, tag="aggrT")
        # scatter windows cover cols [0, W*NW); zero the tail
        nc.gpsimd.memset(aggrT[:, W * NW:NP2], 0.0)

        def mlp_front(ksrc_fn, kch, weights, biases, gs):
            """Layers 1-3, weight-stationary over gs supertiles.

            ksrc_fn(layer, prev) -> list of (get_rhs(g) -> AP, kk).
            Returns the layer-3 h tiles."""
            prev = None
            for layer in range(3):
                ksrc = ksrc_fn(layer, prev)
                cur = []
                for m, (m0, mm) in enumerate(HCH):
                    pss = []
                    for g in range(gs):
                        p = mm_psum.tile([128, ST], f32, tag="mmp")
                        pss.append(p)
                    for k, (get_rhs, kk) in enumerate(ksrc):
                        lhs = weights[layer][k][:, m0:m0 + mm]
                        for g in range(gs):
                            nc.tensor.matmul(pss[g][:mm, :], lhs, get_rhs(g),
                                             start=(k == 0),
                                             stop=(k == len(ksrc) - 1))
                    ht = hpool.tile([HCH[m][1], BST * ST], bf16,
                                    tag=f"h{layer}_{m}")
                    for g in range(gs):
                        if layer >= 1:
                            nc.vector.tensor_scalar(
                                ht[:mm, g * ST:(g + 1) * ST], pss[g][:mm, :],
                                biases[layer][m][:mm, :], 0.0,
                                op0=OP.add, op1=OP.max)
                        else:
                            nc.scalar.activation(
                                ht[:mm, g * ST:(g + 1) * ST], pss[g][:mm, :],
                                AF.Relu, bias=biases[layer][m][:mm, :])
                    cur.append(ht)
                prev = cur
            return prev

        # ---- edge phase ----
        acc_state = {}

        def emit_scatter(blk, msgts):
            for i, mt in enumerate(msgts):
                cidx = blk * 16 + i
                w = cidx // C
                cw = cidx % C
                if cw == 0:
                    at = acc_psum.tile([128, NW], f32, tag="acc")
                    acc_state["t"] = at
                st = spool.tile([128, NW], bf16, tag="st")
                nc.vector.tensor_scalar(
                    st[:, :], iota[:, :], dstloc[:, cidx:cidx + 1], None,
                    op0=OP.is_equal)
                nc.tensor.matmul(acc_state["t"][:, :], mt[:, :], st[:, :],
                                 start=(cw == 0), stop=(cw == C - 1),
                                 skip_group_check=True)
                if cw == C - 1:
                    nc.vector.tensor_copy(
                        aggrT[:, w * NW:(w + 1) * NW], acc_state["t"][:, :])

        def load_block(blk):
            base = blk * (BST * ST)
            tiles = []
            for i, (k0, kk) in enumerate(KIN):
                t = inpool.tile([kk, BST * ST], bf16, tag=f"in{i}")
                nc.sync.dma_start(t[:, :],
                                  d_msg_inT[k0:k0 + kk, base:base + BST * ST])
                tiles.append(t)
            return tiles

        prev_msgts = None
        prev_blk = None
        pending_in = load_block(0)
        for blk in range(NBLK):
            in_t = pending_in
            if blk + 1 < NBLK:
                pending_in = load_block(blk + 1)

            def eksrc(layer, prev):
                if layer == 0:
                    return [
                        ((lambda g, t=t, kk=kk: t[:kk, g * ST:(g + 1) * ST]),
                         kk) for t, (k0, kk) in zip(in_t, KIN)]
                return [
                    ((lambda g, h=prev[i], kk=kk: h[:kk, g * ST:(g + 1) * ST]),
                     kk) for i, (k0, kk) in enumerate(HCH)]

            h3 = mlp_front(eksrc, KIN, mW, mb, BST)

            # L4 weight-stationary: out [MSGD, ST] feature-major
            psl4 = []
            for g in range(BST):
                p4 = mm_psum.tile([128, ST], f32, tag="mmp")
                psl4.append(p4)
            for k, (k0, kk) in enumerate(HCH):
                for g in range(BST):
                    nc.tensor.matmul(psl4[g][:, :], mW[3][k][:kk, :],
                                     h3[k][:kk, g * ST:(g + 1) * ST],
                                     start=(k == 0), stop=(k == 2))
            msgts = []
            for g in range(BST):
                mT = mtpool.tile([128, ST], bf16, tag=f"msgT{g}")
                nc.scalar.activation(mT[:, :], psl4[g][:, :], AF.Identity,
                                     bias=mb4c[:, :])
                for e in range(4):
                    i = g * 4 + e
                    mt = mtpool.tile([128, 128], bf16, tag=f"msgt{i}")
                    nc.sync.dma_start_transpose(
                        mt[:, :], mT[:, e * 128:(e + 1) * 128])
                    msgts.append(mt)

            if prev_msgts is not None:
                emit_scatter(prev_blk, prev_msgts)
            prev_msgts, prev_blk = msgts, blk
        emit_scatter(prev_blk, prev_msgts)

        # ---- node phase ----
        pool_acc = acc_psum.tile([128, NW], f32, tag="acc")
        for t0 in range(0, NT, BST):
            gs = min(BST, NT - t0)

            def nksrc(layer, prev, t0=t0):
                if layer == 0:
                    return [
                        ((lambda g, s=xT, t0=t0:
                          s[:, (t0 + g) * ST:(t0 + g + 1) * ST]), NF),
                        ((lambda g, s=aggrT, t0=t0:
                          s[:, (t0 + g) * ST:(t0 + g + 1) * ST]), MSGD)]
                return [
                    ((lambda g, h=prev[i], kk=kk: h[:kk, g * ST:(g + 1) * ST]),
                     kk) for i, (k0, kk) in enumerate(HCH)]

            h3n = mlp_front(nksrc, None, nW, nb, gs)

            for g in range(gs):
                for e in range(4):
                    tch = (t0 + g) * 4 + e
                    ps = mm_psum.tile([128, ST], f32, tag="mmp")
                    for k, (k0, kk) in enumerate(HCH):
                        nc.tensor.matmul(
                            ps[:, :NF],
                            h3n[k][:kk, g * ST + e * 128:g * ST + (e + 1) * 128],
                            nW[3][k][:kk, :], start=(k == 0), stop=False)
                    nc.tensor.matmul(ps[:, :NF], ones1[:1, :], nb4r[:1, :],
                                     start=False, stop=True)
                    no = mpool.tile([128, NF], bf16, tag="no")
                    nc.scalar.activation(no[:, :], ps[:, :NF], AF.Copy)
                    nc.tensor.matmul(pool_acc[:G, :NF],
                                     pmat[:, tch * G:(tch + 1) * G], no[:, :],
                                     start=(tch == 0), stop=(tch == NCHK - 1),
                                     skip_group_check=True)

        pooled = apool.tile([G, NF], f32, tag="pooled")
        nc.vector.tensor_copy(pooled[:, :], pool_acc[:G, :NF])
        nc.sync.dma_start(d_out[:, :], pooled[:, :])

    nc.compile()
    return nc


def _prep_inputs(x, edge_index, edge_attr, batch, weights, C):
    """Host-side shard/gather/pad. Returns per-core in_maps."""
    E_pad = W * C * 128
    src = np.asarray(edge_index[0], np.int64)
    dst = np.asarray(edge_index[1], np.int64)

    order = np.argsort(dst, kind="stable")
    dsts = dst[order]
    srcs = src[order]

    xT = np.ascontiguousarray(np.asarray(x, np.float32).astype(BF16).T)
    eaT = np.ascontiguousarray(np.asarray(edge_attr, np.float32).astype(BF16).T)
    batch = np.asarray(batch, np.int64)

    bounds = np.searchsorted(dsts, np.arange(0, N_NODES + 1, NPC))

    wcommon = {}
    for i in range(1, 5):
        wcommon[f"mW{i}"] = np.ascontiguousarray(
            weights[f"mW{i}"].astype(BF16))
        wcommon[f"nW{i}"] = np.ascontiguousarray(
            weights[f"nW{i}"].astype(BF16))
    for i in range(1, 4):
        wcommon[f"mb{i}"] = np.ascontiguousarray(
            weights[f"mb{i}"].reshape(HID, 1).astype(np.float32))
        wcommon[f"nb{i}"] = np.ascontiguousarray(
            weights[f"nb{i}"].reshape(HID, 1).astype(np.float32))
    wcommon["mb4c"] = np.ascontiguousarray(
        weights["mb4"].reshape(MSGD, 1).astype(np.float32))
    wcommon["nb4r"] = np.ascontiguousarray(
        weights["nb4"].reshape(1, NF).astype(BF16))

    garange = np.arange(G)
    in_maps = []
    for k in range(NCORES):
        sl = slice(int(bounds[k]), int(bounds[k + 1]))
        eidx = order[sl]
        dloc = dsts[sl] - k * NPC
        srck = srcs[sl]
        win = dloc // NW
        cnt = np.bincount(win, minlength=W)

        starts = np.repeat(np.arange(W) * C * 128, cnt)
        within = np.arange(len(dloc)) - np.repeat(np.cumsum(cnt) - cnt, cnt)
        pos = starts + within

        msg_inT = np.zeros((2 * NF + EF, E_pad), BF16)
        msg_inT[0:NF, pos] = xT[:, k * NPC + dloc]
        msg_inT[NF:2 * NF, pos] = xT[:, srck]
        msg_inT[2 * NF:, pos] = eaT[:, eidx]

        dl = np.full(E_pad, -1.0, np.float32)
        dl[pos] = (dloc - win * NW).astype(np.float32)
        dstloc = np.ascontiguousarray(dl.reshape(E_pad // 128, 128).T)

        xTn = np.zeros((NF, NP2), BF16)
        xTn[:, :NPC] = xT[:, k * NPC:(k + 1) * NPC]

        bl = np.full(NP2, -1, np.int64)
        bl[:NPC] = batch[k * NPC:(k + 1) * NPC]
        P = (bl[:, None] == garange[None, :]).astype(BF16)
        pmat = np.ascontiguousarray(
            P.reshape(NCHK, 128, G).transpose(1, 0, 2).reshape(128, NCHK * G))

        in_map = dict(wcommon)
        in_map.update(msg_inT=msg_inT, dstloc=dstloc, xT=xTn, pmat=pmat)
        in_maps.append(in_map)
    return in_maps


def kernel(**inputs):
    global LAST_EXEC_NS
    from concourse.bass_utils import run_bass_kernel_spmd

    x = np.asarray(inputs["x"], np.float32)
    edge_index = np.asarray(inputs["edge_index"])
    edge_attr = np.asarray(inputs["edge_attr"], np.float32)
    batch = np.asarray(inputs["batch"])

    # chunk count per window from the actual data (uniform across cores)
    dst = np.asarray(edge_index[1], np.int64)
    dloc_all = dst % NPC
    core_all = dst // NPC
    win_all = dloc_all // NW
    cnt = np.bincount(core_all * W + win_all, minlength=NCORES * W)
    C = int(np.ceil(cnt.max() / 128.0))
    C = max(16, int(np.ceil(C / 16.0)) * 16)

    key = C
    if key not in _BUILD_CACHE:
        _BUILD_CACHE[key] = _build_nc(C)
    nc = _BUILD_CACHE[key]

    in_maps = _prep_inputs(x, edge_index, edge_attr, batch, inputs, C)

    res = run_bass_kernel_spmd(nc, in_maps, list(range(NCORES)), trace=TRACE)
    LAST_EXEC_NS = res.exec_time_ns

    total = np.zeros((G, NF), np.float64)
    for r in res.results:
        total += np.asarray(r["partial"], np.float64)

    counts = np.bincount(np.asarray(batch, np.int64), minlength=G)
    pooled = (total / np.maximum(counts, 1)[:, None]).astype(np.float32)
    out = pooled @ np.asarray(inputs["linW"], np.float32) + np.asarray(
        inputs["linb"], np.float32)
    return out.astype(np.float32)


# revision 13
# speedup vs baseline: 1.6319x; 1.0692x over previous
"""GNN message-passing + pooling kernel for 8 Trainium2 NeuronCores.

Strategy (per the sharding hint):
  - Host: sort edges by dst, partition the 50k nodes into 8 contiguous
    ranges of 6250; each core gets the edges targeting its node range
    (disjoint scatter -> no cross-core reduction needed).
  - Host gathers x[dst], x[src], edge_attr into a transposed bf16
    [320, E_pad] tensor per core (edges grouped into 481-node scatter
    windows, padded to a uniform chunk count so the device program is
    identical across cores).
  - Device (per core): 4-layer message MLP in transposed-activation
    layout processed in 2048-edge blocks (4x512 supertiles).  Each
    weight chunk is kept stationary on the PE array for 4 consecutive
    matmuls (amortizes LDWEIGHTS, which otherwise serializes ~100ns per
    matmul).  Layer 4 is computed weight-stationary into a feature-major
    [msg_dim, edges] PSUM tile, bias fused into the PSUM->SBUF copy on
    the scalar engine, then DMA-transposed (xbar) into edge-major
    [128, 128] chunks for the scatter.  Scatter-add via one-hot matmuls
    (one-hot built on DVE with iota + is_equal against per-edge local
    dst), deferred by one block so the transposes are off the critical
    path.  Node MLP over the core's 6250 nodes with the same blocked
    structure, per-graph sum-pooling accumulated in a single PSUM bank.
    Output: [32, 128] partial per-graph sums.
  - Host: sum the 8 partials, divide by per-graph node counts, apply the
    final [128, 16] linear.
"""

import sys

if "/opt/trn_rl_repo" not in sys.path:
    sys.path.insert(0, "/opt/trn_rl_repo")

import numpy as np
import ml_dtypes

BF16 = ml_dtypes.bfloat16

# Problem dims
N_NODES = 50000
N_EDGES = 800000
NF = 128          # node feature dim
EF = 64           # edge feature dim
MSGD = 128        # message dim
HID = 300         # MLP hidden
G = 32            # graphs
NCORES = 8

# Tiling config
NPC = N_NODES // NCORES   # 6250 nodes per core
NW = 481                  # nodes per scatter window
W = 13                    # windows per core (13*481 = 6253 >= 6250)
ST = 512                  # edge supertile (free dim per matmul)
BST = 4                   # supertiles per block (weight-stationary reuse)
NP2 = 6656                # padded nodes per core for node MLP (13*512)
NT = NP2 // ST            # node supertiles
NCHK = NP2 // 128         # node chunks for pooling
HIDP = 384                # HID zero-padded to full 128-row K chunks
KINP = 384                # 2*NF+EF zero-padded likewise

TRACE = False             # set True from test harness to profile core 0
LAST_EXEC_NS = None

_BUILD_CACHE = {}


def _chunks(total, step=128):
    return [(o, min(step, total - o)) for o in range(0, total, step)]


def _build_nc(C):
    """Build the (single) SPMD Bass program. C = 128-edge chunks per window
    (multiple of 16 so each window is a whole number of 2048-edge blocks)."""
    import concourse.bacc as bacc
    import concourse.tile as tile
    from concourse import mybir
    from contextlib import ExitStack

    f32 = mybir.dt.float32
    bf16 = mybir.dt.bfloat16
    AF = mybir.ActivationFunctionType
    OP = mybir.AluOpType

    E_pad = W * C * 128
    NCHUNKS = W * C
    NBLK = NCHUNKS // 16      # 2048-edge blocks

    nc = bacc.Bacc("TRN2", target_bir_lowering=False, debug=False,
                   num_devices=NCORES)

    # --- DRAM I/O ---
    d_msg_inT = nc.dram_tensor("msg_inT", [KINP, E_pad], bf16,
                               kind="ExternalInput")
    d_dstloc = nc.dram_tensor("dstloc", [128, NCHUNKS], f32,
                              kind="ExternalInput")
    d_xT = nc.dram_tensor("xT", [NF, NP2], bf16, kind="ExternalInput")
    d_pmat = nc.dram_tensor("pmat", [128, NCHK * G], bf16,
                            kind="ExternalInput")
    d_mW = [nc.dram_tensor(f"mW{i}", s, bf16, kind="ExternalInput")
            for i, s in enumerate([[KINP, HIDP], [HIDP, HIDP], [HIDP, HIDP],
                                   [HIDP, MSGD]], start=1)]
    d_mb = [nc.dram_tensor(f"mb{i}", [HIDP, 1], f32, kind="ExternalInput")
            for i in range(1, 4)]
    d_mb4c = nc.dram_tensor("mb4c", [MSGD, 1], f32, kind="ExternalInput")
    d_nW = [nc.dram_tensor(f"nW{i}", s, bf16, kind="ExternalInput")
            for i, s in enumerate([[NF + MSGD, HIDP], [HIDP, HIDP],
                                   [HIDP, HIDP], [HIDP, NF]], start=1)]
    d_nb = [nc.dram_tensor(f"nb{i}", [HIDP, 1], f32, kind="ExternalInput")
            for i in range(1, 4)]
    d_nb4r = nc.dram_tensor("nb4r", [1, NF], bf16, kind="ExternalInput")
    d_out = nc.dram_tensor("partial", [G, NF], f32, kind="ExternalOutput")

    HCH = _chunks(HIDP)         # [(0,128),(128,128),(256,128)]
    KIN = _chunks(KINP)         # [(0,128),(128,128),(256,128)]

    with tile.TileContext(nc) as tc, ExitStack() as ctx:
        wpool = ctx.enter_context(tc.tile_pool(name="w", bufs=1))
        apool = ctx.enter_context(tc.tile_pool(name="agg", bufs=1))
        inpool = ctx.enter_context(tc.tile_pool(name="in", bufs=3))
        hpool = ctx.enter_context(tc.tile_pool(name="h", bufs=2))
        mtpool = ctx.enter_context(tc.tile_pool(name="mt", bufs=2))
        mpool = ctx.enter_context(tc.tile_pool(name="m", bufs=4))
        spool = ctx.enter_context(tc.tile_pool(name="s", bufs=8))
        mm_psum = ctx.enter_context(
            tc.tile_pool(name="mmp", bufs=7, space="PSUM"))
        acc_psum = ctx.enter_context(
            tc.tile_pool(name="accp", bufs=1, space="PSUM"))

        def load_w(dram, K, N, dt, name):
            tiles = []
            for i, (k0, kk) in enumerate(_chunks(K)):
                t = wpool.tile([kk, N], dt, tag=f"{name}{i}")
                nc.sync.dma_start(t[:, :], dram[k0:k0 + kk, :])
                tiles.append(t)
            return tiles

        mW = [load_w(d_mW[0], KINP, HIDP, bf16, "mW1"),
              load_w(d_mW[1], HIDP, HIDP, bf16, "mW2"),
              load_w(d_mW[2], HIDP, HIDP, bf16, "mW3"),
              load_w(d_mW[3], HIDP, MSGD, bf16, "mW4")]
        mb = [load_w(d_mb[i], HIDP, 1, f32, f"mb{i + 1}") for i in range(3)]
        nW = [load_w(d_nW[0], NF + MSGD, HIDP, bf16, "nW1"),
              load_w(d_nW[1], HIDP, HIDP, bf16, "nW2"),
              load_w(d_nW[2], HIDP, HIDP, bf16, "nW3"),
              load_w(d_nW[3], HIDP, NF, bf16, "nW4")]
        nb = [load_w(d_nb[i], HIDP, 1, f32, f"nb{i + 1}") for i in range(3)]
        mb4c = wpool.tile([MSGD, 1], f32, tag="mb4c")
        nc.sync.dma_start(mb4c[:, :], d_mb4c[:, :])
        nb4r = wpool.tile([1, NF], bf16, tag="nb4r")
        nc.sync.dma_start(nb4r[:, :], d_nb4r[:, :])

        dstloc = wpool.tile([128, NCHUNKS], f32, tag="dstloc")
        nc.sync.dma_start(dstloc[:, :], d_dstloc[:, :])
        xT = wpool.tile([NF, NP2], bf16, tag="xT")
        nc.sync.dma_start(xT[:, :], d_xT[:, :])
        pmat = wpool.tile([128, NCHK * G], bf16, tag="pmat")
        nc.sync.dma_start(pmat[:, :], d_pmat[:, :])

        iota = wpool.tile([128, NW], f32, tag="iota")
        nc.gpsimd.iota(iota[:, :], pattern=[[1, NW]], base=0,
                       channel_multiplier=0,
                       allow_small_or_imprecise_dtypes=True)
        ones1 = wpool.tile([1, 128], bf16, tag="ones1")
        nc.gpsimd.memset(ones1[:, :], 1.0)

        aggrT = apool.tile([NF, NP2], bf16, tag="aggrT")
        # scatter windows cover cols [0, W*NW); zero the tail
        nc.gpsimd.memset(aggrT[:, W * NW:NP2], 0.0)

        def mlp_front(ksrc_fn, kch, weights, biases, gs):
            """Layers 1-3, weight-stationary over gs supertiles.

            ksrc_fn(layer, prev) -> list of (get_rhs(g) -> AP, kk).
            Returns the layer-3 h tiles."""
            prev = None
            for layer in range(3):
                ksrc = ksrc_fn(layer, prev)
                cur = []
                for m, (m0, mm) in enumerate(HCH):
                    pss = []
                    for g in range(gs):
                        p = mm_psum.tile([128, ST], f32, tag="mmp")
                        pss.append(p)
                    for k, (get_rhs, kk) in enumerate(ksrc):
                        lhs = weights[layer][k][:, m0:m0 + mm]
                        for g in range(gs):
                            nc.tensor.matmul(pss[g][:mm, :], lhs, get_rhs(g),
                                             start=(k == 0),
                                             stop=(k == len(ksrc) - 1))
                    ht = hpool.tile([HCH[m][1], BST * ST], bf16,
                                    tag=f"h{layer}_{m}")
                    for g in range(gs):
                        if layer >= 1:
                            nc.vector.tensor_scalar(
                                ht[:mm, g * ST:(g + 1) * ST], pss[g][:mm, :],
                                biases[layer][m][:mm, :], 0.0,
                                op0=OP.add, op1=OP.max)
                        else:
                            nc.scalar.activation(
                                ht[:mm, g * ST:(g + 1) * ST], pss[g][:mm, :],
                                AF.Relu, bias=biases[layer][m][:mm, :])
                    cur.append(ht)
                prev = cur
            return prev

        # ---- edge phase ----
        acc_state = {}

        def emit_scatter(blk, msgts):
            for i, mt in enumerate(msgts):
                cidx = blk * 16 + i
                w = cidx // C
                cw = cidx % C
                if cw == 0:
                    at = acc_psum.tile([128, NW], f32, tag="acc")
                    acc_state["t"] = at
                st = spool.tile([128, NW], bf16, tag="st")
                nc.vector.tensor_scalar(
                    st[:, :], iota[:, :], dstloc[:, cidx:cidx + 1], None,
                    op0=OP.is_equal)
                nc.tensor.matmul(acc_state["t"][:, :], mt[:, :], st[:, :],
                                 start=(cw == 0), stop=(cw == C - 1),
                                 skip_group_check=True)
                if cw == C - 1:
                    nc.vector.tensor_copy(
                        aggrT[:, w * NW:(w + 1) * NW], acc_state["t"][:, :])

        def load_block(blk):
            base = blk * (BST * ST)
            tiles = []
            for i, (k0, kk) in enumerate(KIN):
                t = inpool.tile([kk, BST * ST], bf16, tag=f"in{i}")
                nc.sync.dma_start(t[:, :],
                                  d_msg_inT[k0:k0 + kk, base:base + BST * ST])
                tiles.append(t)
            return tiles

        prev_msgts = None
        prev_blk = None
        pending_in = load_block(0)
        for blk in range(NBLK):
            in_t = pending_in
            if blk + 1 < NBLK:
                pending_in = load_block(blk + 1)

            def eksrc(layer, prev):
                if layer == 0:
                    return [
                        ((lambda g, t=t, kk=kk: t[:kk, g * ST:(g + 1) * ST]),
                         kk) for t, (k0, kk) in zip(in_t, KIN)]
                return [
                    ((lambda g, h=prev[i], kk=kk: h[:kk, g * ST:(g + 1) * ST]),
                     kk) for i, (k0, kk) in enumerate(HCH)]

            h3 = mlp_front(eksrc, KIN, mW, mb, BST)

            # L4 weight-stationary: out [MSGD, ST] feature-major
            psl4 = []
            for g in range(BST):
                p4 = mm_psum.tile([128, ST], f32, tag="mmp")
                psl4.append(p4)
            for k, (k0, kk) in enumerate(HCH):
                for g in range(BST):
                    nc.tensor.matmul(psl4[g][:, :], mW[3][k][:kk, :],
                                     h3[k][:kk, g * ST:(g + 1) * ST],
                                     start=(k == 0), stop=(k == 2))
            msgts = []
            for g in range(BST):
                mT = mtpool.tile([128, ST], bf16, tag=f"msgT{g}")
                nc.scalar.activation(mT[:, :], psl4[g][:, :], AF.Identity,
                                     bias=mb4c[:, :])
                for e in range(4):
                    i = g * 4 + e
                    mt = mtpool.tile([128, 128], bf16, tag=f"msgt{i}")
                    nc.sync.dma_start_transpose(
                        mt[:, :], mT[:, e * 128:(e + 1) * 128])
                    msgts.append(mt)

            if prev_msgts is not None:
                emit_scatter(prev_blk, prev_msgts)
            prev_msgts, prev_blk = msgts, blk
        emit_scatter(prev_blk, prev_msgts)

        # ---- node phase ----
        pool_acc = acc_psum.tile([128, NW], f32, tag="acc")
        for t0 in range(0, NT, BST):
            gs = min(BST, NT - t0)

            def nksrc(layer, prev, t0=t0):
                if layer == 0:
                    return [
                        ((lambda g, s=xT, t0=t0:
                          s[:, (t0 + g) * ST:(t0 + g + 1) * ST]), NF),
                        ((lambda g, s=aggrT, t0=t0:
                          s[:, (t0 + g) * ST:(t0 + g + 1) * ST]), MSGD)]
                return [
                    ((lambda g, h=prev[i], kk=kk: h[:kk, g * ST:(g + 1) * ST]),
                     kk) for i, (k0, kk) in enumerate(HCH)]

            h3n = mlp_front(nksrc, None, nW, nb, gs)

            for g in range(gs):
                for e in range(4):
                    tch = (t0 + g) * 4 + e
                    ps = mm_psum.tile([128, ST], f32, tag="mmp")
                    for k, (k0, kk) in enumerate(HCH):
                        nc.tensor.matmul(
                            ps[:, :NF],
                            h3n[k][:kk, g * ST + e * 128:g * ST + (e + 1) * 128],
                            nW[3][k][:kk, :], start=(k == 0), stop=False)
                    nc.tensor.matmul(ps[:, :NF], ones1[:1, :], nb4r[:1, :],
                                     start=False, stop=True)
                    no = mpool.tile([128, NF], bf16, tag="no")
                    nc.scalar.activation(no[:, :], ps[:, :NF], AF.Copy)
                    nc.tensor.matmul(pool_acc[:G, :NF],
                                     pmat[:, tch * G:(tch + 1) * G], no[:, :],
                                     start=(tch == 0), stop=(tch == NCHK - 1),
                                     skip_group_check=True)

        pooled = apool.tile([G, NF], f32, tag="pooled")
        nc.vector.tensor_copy(pooled[:, :], pool_acc[:G, :NF])
        nc.sync.dma_start(d_out[:, :], pooled[:, :])

    nc.compile()
    return nc


def _prep_inputs(x, edge_index, edge_attr, batch, weights, C):
    """Host-side shard/gather/pad. Returns per-core in_maps."""
    E_pad = W * C * 128
    src = np.asarray(edge_index[0], np.int64)
    dst = np.asarray(edge_index[1], np.int64)

    order = np.argsort(dst, kind="stable")
    dsts = dst[order]
    srcs = src[order]

    xT = np.ascontiguousarray(np.asarray(x, np.float32).astype(BF16).T)
    eaT = np.ascontiguousarray(np.asarray(edge_attr, np.float32).astype(BF16).T)
    batch = np.asarray(batch, np.int64)

    bounds = np.searchsorted(dsts, np.arange(0, N_NODES + 1, NPC))

    def pad2(a, r, c):
        out = np.zeros((r, c), a.dtype)
        out[:a.shape[0], :a.shape[1]] = a
        return out

    wcommon = {}
    for i in range(1, 5):
        mw = weights[f"mW{i}"].astype(BF16)
        nw = weights[f"nW{i}"].astype(BF16)
        kr = KINP if i == 1 else HIDP
        kc = MSGD if i == 4 else HIDP
        wcommon[f"mW{i}"] = pad2(mw, kr, kc)
        nr = NF + MSGD if i == 1 else HIDP
        ncol = NF if i == 4 else HIDP
        wcommon[f"nW{i}"] = pad2(nw, nr, ncol)
    for i in range(1, 4):
        wcommon[f"mb{i}"] = pad2(
            weights[f"mb{i}"].reshape(HID, 1).astype(np.float32), HIDP, 1)
        wcommon[f"nb{i}"] = pad2(
            weights[f"nb{i}"].reshape(HID, 1).astype(np.float32), HIDP, 1)
    wcommon["mb4c"] = np.ascontiguousarray(
        weights["mb4"].reshape(MSGD, 1).astype(np.float32))
    wcommon["nb4r"] = np.ascontiguousarray(
        weights["nb4"].reshape(1, NF).astype(BF16))

    garange = np.arange(G)
    in_maps = []
    for k in range(NCORES):
        sl = slice(int(bounds[k]), int(bounds[k + 1]))
        eidx = order[sl]
        dloc = dsts[sl] - k * NPC
        srck = srcs[sl]
        win = dloc // NW
        cnt = np.bincount(win, minlength=W)

        starts = np.repeat(np.arange(W) * C * 128, cnt)
        within = np.arange(len(dloc)) - np.repeat(np.cumsum(cnt) - cnt, cnt)
        pos = starts + within

        msg_inT = np.zeros((KINP, E_pad), BF16)
        msg_inT[0:NF, pos] = xT[:, k * NPC + dloc]
        msg_inT[NF:2 * NF, pos] = xT[:, srck]
        msg_inT[2 * NF:2 * NF + EF, pos] = eaT[:, eidx]

        dl = np.full(E_pad, -1.0, np.float32)
        dl[pos] = (dloc - win * NW).astype(np.float32)
        dstloc = np.ascontiguousarray(dl.reshape(E_pad // 128, 128).T)

        xTn = np.zeros((NF, NP2), BF16)
        xTn[:, :NPC] = xT[:, k * NPC:(k + 1) * NPC]

        bl = np.full(NP2, -1, np.int64)
        bl[:NPC] = batch[k * NPC:(k + 1) * NPC]
        P = (bl[:, None] == garange[None, :]).astype(BF16)
        pmat = np.ascontiguousarray(
            P.reshape(NCHK, 128, G).transpose(1, 0, 2).reshape(128, NCHK * G))

        in_map = dict(wcommon)
        in_map.update(msg_inT=msg_inT, dstloc=dstloc, xT=xTn, pmat=pmat)
        in_maps.append(in_map)
    return in_maps


def kernel(**inputs):
    global LAST_EXEC_NS
    from concourse.bass_utils import run_bass_kernel_spmd

    x = np.asarray(inputs["x"], np.float32)
    edge_index = np.asarray(inputs["edge_index"])
    edge_attr = np.asarray(inputs["edge_attr"], np.float32)
    batch = np.asarray(inputs["batch"])

    # chunk count per window from the actual data (uniform across cores)
    dst = np.asarray(edge_index[1], np.int64)
    dloc_all = dst % NPC
    core_all = dst // NPC
    win_all = dloc_all // NW
    cnt = np.bincount(core_all * W + win_all, minlength=NCORES * W)
    C = int(np.ceil(cnt.max() / 128.0))
    C = max(16, int(np.ceil(C / 16.0)) * 16)

    key = C
    if key not in _BUILD_CACHE:
        _BUILD_CACHE[key] = _build_nc(C)
    nc = _BUILD_CACHE[key]

    in_maps = _prep_inputs(x, edge_index, edge_attr, batch, inputs, C)

    res = run_bass_kernel_spmd(nc, in_maps, list(range(NCORES)), trace=TRACE)
    LAST_EXEC_NS = res.exec_time_ns

    total = np.zeros((G, NF), np.float64)
    for r in res.results:
        total += np.asarray(r["partial"], np.float64)

    counts = np.bincount(np.asarray(batch, np.int64), minlength=G)
    pooled = (total / np.maximum(counts, 1)[:, None]).astype(np.float32)
    out = pooled @ np.asarray(inputs["linW"], np.float32) + np.asarray(
        inputs["linb"], np.float32)
    return out.astype(np.float32)


# revision 15
# speedup vs baseline: 1.6497x; 1.0109x over previous
"""GNN message-passing + pooling kernel for 8 Trainium2 NeuronCores.

Strategy (per the sharding hint):
  - Host: sort edges by dst, partition the 50k nodes into 8 contiguous
    ranges of 6250; each core gets the edges targeting its node range
    (disjoint scatter -> no cross-core reduction needed).
  - Host gathers x[dst], x[src], edge_attr into a transposed bf16
    [320, E_pad] tensor per core (edges grouped into 481-node scatter
    windows, padded to a uniform chunk count so the device program is
    identical across cores).
  - Device (per core): 4-layer message MLP in transposed-activation
    layout processed in 2048-edge blocks (4x512 supertiles).  Each
    weight chunk is kept stationary on the PE array for 4 consecutive
    matmuls (amortizes LDWEIGHTS, which otherwise serializes ~100ns per
    matmul).  Layer 4 is computed weight-stationary into a feature-major
    [msg_dim, edges] PSUM tile, bias fused into the PSUM->SBUF copy on
    the scalar engine, then DMA-transposed (xbar) into edge-major
    [128, 128] chunks for the scatter.  Scatter-add via one-hot matmuls
    (one-hot built on DVE with iota + is_equal against per-edge local
    dst), deferred by one block so the transposes are off the critical
    path.  Node MLP over the core's 6250 nodes with the same blocked
    structure, per-graph sum-pooling accumulated in a single PSUM bank.
    Output: [32, 128] partial per-graph sums.
  - Host: sum the 8 partials, divide by per-graph node counts, apply the
    final [128, 16] linear.
"""

import sys

if "/opt/trn_rl_repo" not in sys.path:
    sys.path.insert(0, "/opt/trn_rl_repo")

import numpy as np
import ml_dtypes

BF16 = ml_dtypes.bfloat16

# Problem dims
N_NODES = 50000
N_EDGES = 800000
NF = 128          # node feature dim
EF = 64           # edge feature dim
MSGD = 128        # message dim
HID = 300         # MLP hidden
G = 32            # graphs
NCORES = 8

# Tiling config
NPC = N_NODES // NCORES   # 6250 nodes per core
NW = 241                  # nodes per scatter window
W = 26                    # windows per core (26*241 = 6266 >= 6250)
ST = 512                  # edge supertile (free dim per matmul)
BST = 4                   # supertiles per block (weight-stationary reuse)
NP2 = 6656                # padded nodes per core for node MLP (13*512)
NT = NP2 // ST            # node supertiles
NCHK = NP2 // 128         # node chunks for pooling
HIDP = 384                # HID zero-padded to full 128-row K chunks
KINP = 384                # 2*NF+EF zero-padded likewise

TRACE = False             # set True from test harness to profile core 0
LAST_EXEC_NS = None

_BUILD_CACHE = {}


def _chunks(total, step=128):
    return [(o, min(step, total - o)) for o in range(0, total, step)]


def _build_nc(C):
    """Build the (single) SPMD Bass program. C = 128-edge chunks per window
    (multiple of 16 so each window is a whole number of 2048-edge blocks)."""
    import concourse.bacc as bacc
    import concourse.tile as tile
    from concourse import mybir
    from contextlib import ExitStack

    f32 = mybir.dt.float32
    bf16 = mybir.dt.bfloat16
    AF = mybir.ActivationFunctionType
    OP = mybir.AluOpType

    E_pad = W * C * 128
    NCHUNKS = W * C
    NBLK = NCHUNKS // 16      # 2048-edge blocks

    nc = bacc.Bacc("TRN2", target_bir_lowering=False, debug=False,
                   num_devices=NCORES)

    # --- DRAM I/O ---
    d_msg_inT = nc.dram_tensor("msg_inT", [KINP, E_pad], bf16,
                               kind="ExternalInput")
    d_dstloc = nc.dram_tensor("dstloc", [128, NCHUNKS], f32,
                              kind="ExternalInput")
    d_xT = nc.dram_tensor("xT", [NF, NP2], bf16, kind="ExternalInput")
    d_pmat = nc.dram_tensor("pmat", [128, NCHK * G], bf16,
                            kind="ExternalInput")
    d_mW = [nc.dram_tensor(f"mW{i}", s, bf16, kind="ExternalInput")
            for i, s in enumerate([[KINP, HIDP], [HIDP, HIDP], [HIDP, HIDP],
                                   [HIDP, MSGD]], start=1)]
    d_mb = [nc.dram_tensor(f"mb{i}", [HIDP, 1], f32, kind="ExternalInput")
            for i in range(1, 4)]
    d_mb4c = nc.dram_tensor("mb4c", [MSGD, 1], f32, kind="ExternalInput")
    d_nW = [nc.dram_tensor(f"nW{i}", s, bf16, kind="ExternalInput")
            for i, s in enumerate([[NF + MSGD, HIDP], [HIDP, HIDP],
                                   [HIDP, HIDP], [HIDP, NF]], start=1)]
    d_nb = [nc.dram_tensor(f"nb{i}", [HIDP, 1], f32, kind="ExternalInput")
            for i in range(1, 4)]
    d_nb4r = nc.dram_tensor("nb4r", [1, NF], bf16, kind="ExternalInput")
    d_out = nc.dram_tensor("partial", [G, NF], f32, kind="ExternalOutput")

    HCH = _chunks(HIDP)         # [(0,128),(128,128),(256,128)]
    KIN = _chunks(KINP)         # [(0,128),(128,128),(256,128)]

    with tile.TileContext(nc) as tc, ExitStack() as ctx:
        wpool = ctx.enter_context(tc.tile_pool(name="w", bufs=1))
        apool = ctx.enter_context(tc.tile_pool(name="agg", bufs=1))
        inpool = ctx.enter_context(tc.tile_pool(name="in", bufs=3))
        hpool = ctx.enter_context(tc.tile_pool(name="h", bufs=2))
        mtpool = ctx.enter_context(tc.tile_pool(name="mt", bufs=2))
        mpool = ctx.enter_context(tc.tile_pool(name="m", bufs=4))
        spool = ctx.enter_context(tc.tile_pool(name="s", bufs=8))
        mm_psum = ctx.enter_context(
            tc.tile_pool(name="mmp", bufs=7, space="PSUM"))
        acc_psum = ctx.enter_context(
            tc.tile_pool(name="accp", bufs=1, space="PSUM"))

        def load_w(dram, K, N, dt, name):
            tiles = []
            for i, (k0, kk) in enumerate(_chunks(K)):
                t = wpool.tile([kk, N], dt, tag=f"{name}{i}")
                nc.sync.dma_start(t[:, :], dram[k0:k0 + kk, :])
                tiles.append(t)
            return tiles

        mW = [load_w(d_mW[0], KINP, HIDP, bf16, "mW1"),
              load_w(d_mW[1], HIDP, HIDP, bf16, "mW2"),
              load_w(d_mW[2], HIDP, HIDP, bf16, "mW3"),
              load_w(d_mW[3], HIDP, MSGD, bf16, "mW4")]
        mb = [load_w(d_mb[i], HIDP, 1, f32, f"mb{i + 1}") for i in range(3)]
        nW = [load_w(d_nW[0], NF + MSGD, HIDP, bf16, "nW1"),
              load_w(d_nW[1], HIDP, HIDP, bf16, "nW2"),
              load_w(d_nW[2], HIDP, HIDP, bf16, "nW3"),
              load_w(d_nW[3], HIDP, NF, bf16, "nW4")]
        nb = [load_w(d_nb[i], HIDP, 1, f32, f"nb{i + 1}") for i in range(3)]
        mb4c = wpool.tile([MSGD, 1], f32, tag="mb4c")
        nc.sync.dma_start(mb4c[:, :], d_mb4c[:, :])
        nb4r = wpool.tile([1, NF], bf16, tag="nb4r")
        nc.sync.dma_start(nb4r[:, :], d_nb4r[:, :])

        dstloc = wpool.tile([128, NCHUNKS], f32, tag="dstloc")
        nc.sync.dma_start(dstloc[:, :], d_dstloc[:, :])
        xT = wpool.tile([NF, NP2], bf16, tag="xT")
        nc.sync.dma_start(xT[:, :], d_xT[:, :])
        pmat = wpool.tile([128, NCHK * G], bf16, tag="pmat")
        nc.sync.dma_start(pmat[:, :], d_pmat[:, :])

        iota = wpool.tile([128, NW], f32, tag="iota")
        nc.gpsimd.iota(iota[:, :], pattern=[[1, NW]], base=0,
                       channel_multiplier=0,
                       allow_small_or_imprecise_dtypes=True)
        ones1 = wpool.tile([1, 128], bf16, tag="ones1")
        nc.gpsimd.memset(ones1[:, :], 1.0)

        aggrT = apool.tile([NF, NP2], bf16, tag="aggrT")
        # scatter windows cover cols [0, W*NW); zero the tail
        nc.gpsimd.memset(aggrT[:, W * NW:NP2], 0.0)

        def mlp_front(ksrc_fn, kch, weights, biases, gs):
            """Layers 1-3, weight-stationary over gs supertiles.

            ksrc_fn(layer, prev) -> list of (get_rhs(g) -> AP, kk).
            Returns the layer-3 h tiles."""
            prev = None
            for layer in range(3):
                ksrc = ksrc_fn(layer, prev)
                cur = []
                for m, (m0, mm) in enumerate(HCH):
                    pss = []
                    for g in range(gs):
                        p = mm_psum.tile([128, ST], f32, tag="mmp")
                        pss.append(p)
                    for k, (get_rhs, kk) in enumerate(ksrc):
                        lhs = weights[layer][k][:, m0:m0 + mm]
                        for g in range(gs):
                            nc.tensor.matmul(pss[g][:mm, :], lhs, get_rhs(g),
                                             start=(k == 0),
                                             stop=(k == len(ksrc) - 1))
                    ht = hpool.tile([HCH[m][1], BST * ST], bf16,
                                    tag=f"h{layer}_{m}")
                    for g in range(gs):
                        if layer >= 1:
                            nc.vector.tensor_scalar(
                                ht[:mm, g * ST:(g + 1) * ST], pss[g][:mm, :],
                                biases[layer][m][:mm, :], 0.0,
                                op0=OP.add, op1=OP.max)
                        else:
                            nc.scalar.activation(
                                ht[:mm, g * ST:(g + 1) * ST], pss[g][:mm, :],
                                AF.Relu, bias=biases[layer][m][:mm, :])
                    cur.append(ht)
                prev = cur
            return prev

        # ---- edge phase ----
        acc_state = {}

        def emit_scatter(blk, msgts):
            for i, mt in enumerate(msgts):
                cidx = blk * 16 + i
                w = cidx // C
                cw = cidx % C
                if cw == 0:
                    at = acc_psum.tile([128, NW], f32, tag="acc")
                    acc_state["t"] = at
                st = spool.tile([128, NW], bf16, tag="st")
                nc.vector.tensor_scalar(
                    st[:, :], iota[:, :], dstloc[:, cidx:cidx + 1], None,
                    op0=OP.is_equal)
                nc.tensor.matmul(acc_state["t"][:, :], mt[:, :], st[:, :],
                                 start=(cw == 0), stop=(cw == C - 1),
                                 skip_group_check=True)
                if cw == C - 1:
                    nc.vector.tensor_copy(
                        aggrT[:, w * NW:(w + 1) * NW], acc_state["t"][:, :])

        def load_block(blk):
            base = blk * (BST * ST)
            tiles = []
            for i, (k0, kk) in enumerate(KIN):
                t = inpool.tile([kk, BST * ST], bf16, tag=f"in{i}")
                nc.sync.dma_start(t[:, :],
                                  d_msg_inT[k0:k0 + kk, base:base + BST * ST])
                tiles.append(t)
            return tiles

        prev_msgts = None
        prev_blk = None
        pending_in = load_block(0)
        for blk in range(NBLK):
            in_t = pending_in
            if blk + 1 < NBLK:
                pending_in = load_block(blk + 1)

            def eksrc(layer, prev):
                if layer == 0:
                    return [
                        ((lambda g, t=t, kk=kk: t[:kk, g * ST:(g + 1) * ST]),
                         kk) for t, (k0, kk) in zip(in_t, KIN)]
                return [
                    ((lambda g, h=prev[i], kk=kk: h[:kk, g * ST:(g + 1) * ST]),
                     kk) for i, (k0, kk) in enumerate(HCH)]

            h3 = mlp_front(eksrc, KIN, mW, mb, BST)

            # L4 weight-stationary: out [MSGD, ST] feature-major
            psl4 = []
            for g in range(BST):
                p4 = mm_psum.tile([128, ST], f32, tag="mmp")
                psl4.append(p4)
            for k, (k0, kk) in enumerate(HCH):
                for g in range(BST):
                    nc.tensor.matmul(psl4[g][:, :], mW[3][k][:kk, :],
                                     h3[k][:kk, g * ST:(g + 1) * ST],
                                     start=(k == 0), stop=(k == 2))
            msgts = []
            for g in range(BST):
                mT = mtpool.tile([128, ST], bf16, tag=f"msgT{g}")
                nc.scalar.activation(mT[:, :], psl4[g][:, :], AF.Identity,
                                     bias=mb4c[:, :])
                for e in range(4):
                    i = g * 4 + e
                    mt = mtpool.tile([128, 128], bf16, tag=f"msgt{i}")
                    nc.sync.dma_start_transpose(
                        mt[:, :], mT[:, e * 128:(e + 1) * 128])
                    msgts.append(mt)

            if prev_msgts is not None:
                emit_scatter(prev_blk, prev_msgts)
            prev_msgts, prev_blk = msgts, blk
        emit_scatter(prev_blk, prev_msgts)

        # ---- node phase ----
        pool_acc = acc_psum.tile([128, NW], f32, tag="acc")
        for t0 in range(0, NT, BST):
            gs = min(BST, NT - t0)

            def nksrc(layer, prev, t0=t0):
                if layer == 0:
                    return [
                        ((lambda g, s=xT, t0=t0:
                          s[:, (t0 + g) * ST:(t0 + g + 1) * ST]), NF),
                        ((lambda g, s=aggrT, t0=t0:
                          s[:, (t0 + g) * ST:(t0 + g + 1) * ST]), MSGD)]
                return [
                    ((lambda g, h=prev[i], kk=kk: h[:kk, g * ST:(g + 1) * ST]),
                     kk) for i, (k0, kk) in enumerate(HCH)]

            h3n = mlp_front(nksrc, None, nW, nb, gs)

            for g in range(gs):
                for e in range(4):
                    tch = (t0 + g) * 4 + e
                    ps = mm_psum.tile([128, ST], f32, tag="mmp")
                    for k, (k0, kk) in enumerate(HCH):
                        nc.tensor.matmul(
                            ps[:, :NF],
                            h3n[k][:kk, g * ST + e * 128:g * ST + (e + 1) * 128],
                            nW[3][k][:kk, :], start=(k == 0), stop=False)
                    nc.tensor.matmul(ps[:, :NF], ones1[:1, :], nb4r[:1, :],
                                     start=False, stop=True)
                    no = mpool.tile([128, NF], bf16, tag="no")
                    nc.scalar.activation(no[:, :], ps[:, :NF], AF.Copy)
                    nc.tensor.matmul(pool_acc[:G, :NF],
                                     pmat[:, tch * G:(tch + 1) * G], no[:, :],
                                     start=(tch == 0), stop=(tch == NCHK - 1),
                                     skip_group_check=True)

        pooled = apool.tile([G, NF], f32, tag="pooled")
        nc.vector.tensor_copy(pooled[:, :], pool_acc[:G, :NF])
        nc.sync.dma_start(d_out[:, :], pooled[:, :])

    nc.compile()
    return nc


def _prep_inputs(x, edge_index, edge_attr, batch, weights, C):
    """Host-side shard/gather/pad. Returns per-core in_maps."""
    E_pad = W * C * 128
    src = np.asarray(edge_index[0], np.int64)
    dst = np.asarray(edge_index[1], np.int64)

    order = np.argsort(dst, kind="stable")
    dsts = dst[order]
    srcs = src[order]

    xT = np.ascontiguousarray(np.asarray(x, np.float32).astype(BF16).T)
    eaT = np.ascontiguousarray(np.asarray(edge_attr, np.float32).astype(BF16).T)
    batch = np.asarray(batch, np.int64)

    bounds = np.searchsorted(dsts, np.arange(0, N_NODES + 1, NPC))

    def pad2(a, r, c):
        out = np.zeros((r, c), a.dtype)
        out[:a.shape[0], :a.shape[1]] = a
        return out

    wcommon = {}
    for i in range(1, 5):
        mw = weights[f"mW{i}"].astype(BF16)
        nw = weights[f"nW{i}"].astype(BF16)
        kr = KINP if i == 1 else HIDP
        kc = MSGD if i == 4 else HIDP
        wcommon[f"mW{i}"] = pad2(mw, kr, kc)
        nr = NF + MSGD if i == 1 else HIDP
        ncol = NF if i == 4 else HIDP
        wcommon[f"nW{i}"] = pad2(nw, nr, ncol)
    for i in range(1, 4):
        wcommon[f"mb{i}"] = pad2(
            weights[f"mb{i}"].reshape(HID, 1).astype(np.float32), HIDP, 1)
        wcommon[f"nb{i}"] = pad2(
            weights[f"nb{i}"].reshape(HID, 1).astype(np.float32), HIDP, 1)
    wcommon["mb4c"] = np.ascontiguousarray(
        weights["mb4"].reshape(MSGD, 1).astype(np.float32))
    wcommon["nb4r"] = np.ascontiguousarray(
        weights["nb4"].reshape(1, NF).astype(BF16))

    garange = np.arange(G)
    in_maps = []
    for k in range(NCORES):
        sl = slice(int(bounds[k]), int(bounds[k + 1]))
        eidx = order[sl]
        dloc = dsts[sl] - k * NPC
        srck = srcs[sl]
        win = dloc // NW
        cnt = np.bincount(win, minlength=W)

        starts = np.repeat(np.arange(W) * C * 128, cnt)
        within = np.arange(len(dloc)) - np.repeat(np.cumsum(cnt) - cnt, cnt)
        pos = starts + within

        msg_inT = np.zeros((KINP, E_pad), BF16)
        msg_inT[0:NF, pos] = xT[:, k * NPC + dloc]
        msg_inT[NF:2 * NF, pos] = xT[:, srck]
        msg_inT[2 * NF:2 * NF + EF, pos] = eaT[:, eidx]

        dl = np.full(E_pad, -1.0, np.float32)
        dl[pos] = (dloc - win * NW).astype(np.float32)
        dstloc = np.ascontiguousarray(dl.reshape(E_pad // 128, 128).T)

        xTn = np.zeros((NF, NP2), BF16)
        xTn[:, :NPC] = xT[:, k * NPC:(k + 1) * NPC]

        bl = np.full(NP2, -1, np.int64)
        bl[:NPC] = batch[k * NPC:(k + 1) * NPC]
        P = (bl[:, None] == garange[None, :]).astype(BF16)
        pmat = np.ascontiguousarray(
            P.reshape(NCHK, 128, G).transpose(1, 0, 2).reshape(128, NCHK * G))

        in_map = dict(wcommon)
        in_map.update(msg_inT=msg_inT, dstloc=dstloc, xT=xTn, pmat=pmat)
        in_maps.append(in_map)
    return in_maps


def kernel(**inputs):
    global LAST_EXEC_NS
    from concourse.bass_utils import run_bass_kernel_spmd

    x = np.asarray(inputs["x"], np.float32)
    edge_index = np.asarray(inputs["edge_index"])
    edge_attr = np.asarray(inputs["edge_attr"], np.float32)
    batch = np.asarray(inputs["batch"])

    # chunk count per window from the actual data (uniform across cores)
    dst = np.asarray(edge_index[1], np.int64)
    dloc_all = dst % NPC
    core_all = dst // NPC
    win_all = dloc_all // NW
    cnt = np.bincount(core_all * W + win_all, minlength=NCORES * W)
    C = int(np.ceil(cnt.max() / 128.0))
    C = max(C, 8)
    while (W * C) % 16 != 0:
        C += 1

    key = C
    if key not in _BUILD_CACHE:
        _BUILD_CACHE[key] = _build_nc(C)
    nc = _BUILD_CACHE[key]

    in_maps = _prep_inputs(x, edge_index, edge_attr, batch, inputs, C)

    res = run_bass_kernel_spmd(nc, in_maps, list(range(NCORES)), trace=TRACE)
    LAST_EXEC_NS = res.exec_time_ns

    total = np.zeros((G, NF), np.float64)
    for r in res.results:
        total += np.asarray(r["partial"], np.float64)

    counts = np.bincount(np.asarray(batch, np.int64), minlength=G)
    pooled = (total / np.maximum(counts, 1)[:, None]).astype(np.float32)
    out = pooled @ np.asarray(inputs["linW"], np.float32) + np.asarray(
        inputs["linb"], np.float32)
    return out.astype(np.float32)


# revision 19
# speedup vs baseline: 1.7661x; 1.0706x over previous
"""GNN message-passing + pooling kernel for 8 Trainium2 NeuronCores.

Strategy (per the sharding hint):
  - Host: sort edges by dst, partition the 50k nodes into 8 contiguous
    ranges of 6250; each core gets the edges targeting its node range
    (disjoint scatter -> no cross-core reduction needed).
  - Host gathers x[dst], x[src], edge_attr into a transposed bf16
    [320, E_pad] tensor per core (edges grouped into 481-node scatter
    windows, padded to a uniform chunk count so the device program is
    identical across cores).
  - Device (per core): 4-layer message MLP in transposed-activation
    layout processed in 2048-edge blocks (4x512 supertiles).  Each
    weight chunk is kept stationary on the PE array for 4 consecutive
    matmuls (amortizes LDWEIGHTS, which otherwise serializes ~100ns per
    matmul).  Layer 4 is computed weight-stationary into a feature-major
    [msg_dim, edges] PSUM tile, bias fused into the PSUM->SBUF copy on
    the scalar engine, then DMA-transposed (xbar) into edge-major
    [128, 128] chunks for the scatter.  Scatter-add via one-hot matmuls
    (one-hot built on DVE with iota + is_equal against per-edge local
    dst), deferred by one block so the transposes are off the critical
    path.  Node MLP over the core's 6250 nodes with the same blocked
    structure, per-graph sum-pooling accumulated in a single PSUM bank.
    Output: [32, 128] partial per-graph sums.
  - Host: sum the 8 partials, divide by per-graph node counts, apply the
    final [128, 16] linear.
"""

import sys

if "/opt/trn_rl_repo" not in sys.path:
    sys.path.insert(0, "/opt/trn_rl_repo")

import numpy as np
import ml_dtypes

BF16 = ml_dtypes.bfloat16

# Problem dims
N_NODES = 50000
N_EDGES = 800000
NF = 128          # node feature dim
EF = 64           # edge feature dim
MSGD = 128        # message dim
HID = 300         # MLP hidden
G = 32            # graphs
NCORES = 8

# Tiling config
NPC = N_NODES // NCORES   # 6250 nodes per core
NW = 241                  # nodes per scatter window
W = 26                    # windows per core (26*241 = 6266 >= 6250)
ST = 512                  # edge supertile (free dim per matmul)
BST = 4                   # supertiles per block (weight-stationary reuse)
NP2 = 6656                # padded nodes per core for node MLP (13*512)
NT = NP2 // ST            # node supertiles
NCHK = NP2 // 128         # node chunks for pooling
HIDP = 384                # HID zero-padded to full 128-row K chunks
KINP = 384                # 2*NF+EF zero-padded likewise

TRACE = False             # set True from test harness to profile core 0
LAST_EXEC_NS = None

_BUILD_CACHE = {}


def _chunks(total, step=128):
    return [(o, min(step, total - o)) for o in range(0, total, step)]


def _build_nc(C):
    """Build the (single) SPMD Bass program. C = 128-edge chunks per window
    (multiple of 16 so each window is a whole number of 2048-edge blocks)."""
    import concourse.bacc as bacc
    import concourse.tile as tile
    from concourse import mybir
    from contextlib import ExitStack

    f32 = mybir.dt.float32
    bf16 = mybir.dt.bfloat16
    AF = mybir.ActivationFunctionType
    OP = mybir.AluOpType

    E_pad = W * C * 128
    NCHUNKS = W * C
    NBLK = NCHUNKS // 16      # 2048-edge blocks

    nc = bacc.Bacc("TRN2", target_bir_lowering=False, debug=False,
                   num_devices=NCORES)

    # --- DRAM I/O ---
    d_msg_inT = nc.dram_tensor("msg_inT", [KINP, E_pad], bf16,
                               kind="ExternalInput")
    d_dstloc = nc.dram_tensor("dstloc", [128, NCHUNKS], f32,
                              kind="ExternalInput")
    d_xT = nc.dram_tensor("xT", [NF, NP2], bf16, kind="ExternalInput")
    d_pmat = nc.dram_tensor("pmat", [128, NCHK * G], bf16,
                            kind="ExternalInput")
    d_mW = [nc.dram_tensor(f"mW{i}", s, bf16, kind="ExternalInput")
            for i, s in enumerate([[KINP, HIDP], [HIDP, HIDP], [HIDP, HIDP],
                                   [HIDP, MSGD]], start=1)]
    d_mb = [nc.dram_tensor(f"mb{i}", [HIDP, 1], f32, kind="ExternalInput")
            for i in range(1, 4)]
    d_mb4c = nc.dram_tensor("mb4c", [MSGD, 1], f32, kind="ExternalInput")
    d_nW = [nc.dram_tensor(f"nW{i}", s, bf16, kind="ExternalInput")
            for i, s in enumerate([[NF + MSGD, HIDP], [HIDP, HIDP],
                                   [HIDP, HIDP], [HIDP, NF]], start=1)]
    d_nb = [nc.dram_tensor(f"nb{i}", [HIDP, 1], f32, kind="ExternalInput")
            for i in range(1, 4)]
    d_nb4r = nc.dram_tensor("nb4r", [1, NF], bf16, kind="ExternalInput")
    d_out = nc.dram_tensor("partial", [G, NF], f32, kind="ExternalOutput")

    HCH = _chunks(HIDP)         # [(0,128),(128,128),(256,128)]
    KIN = _chunks(KINP)         # [(0,128),(128,128),(256,128)]

    with tile.TileContext(nc) as tc, ExitStack() as ctx:
        wpool = ctx.enter_context(tc.tile_pool(name="w", bufs=1))
        apool = ctx.enter_context(tc.tile_pool(name="agg", bufs=1))
        inpool = ctx.enter_context(tc.tile_pool(name="in", bufs=3))
        hpool = ctx.enter_context(tc.tile_pool(name="h", bufs=2))
        mtpool = ctx.enter_context(tc.tile_pool(name="mt", bufs=2))
        mpool = ctx.enter_context(tc.tile_pool(name="m", bufs=4))
        spool = ctx.enter_context(tc.tile_pool(name="s", bufs=20))
        mm_psum = ctx.enter_context(
            tc.tile_pool(name="mmp", bufs=7, space="PSUM"))
        acc_psum = ctx.enter_context(
            tc.tile_pool(name="accp", bufs=1, space="PSUM"))

        def load_w(dram, K, N, dt, name):
            tiles = []
            for i, (k0, kk) in enumerate(_chunks(K)):
                t = wpool.tile([kk, N], dt, tag=f"{name}{i}")
                nc.sync.dma_start(t[:, :], dram[k0:k0 + kk, :])
                tiles.append(t)
            return tiles

        mW = [load_w(d_mW[0], KINP, HIDP, bf16, "mW1"),
              load_w(d_mW[1], HIDP, HIDP, bf16, "mW2"),
              load_w(d_mW[2], HIDP, HIDP, bf16, "mW3"),
              load_w(d_mW[3], HIDP, MSGD, bf16, "mW4")]
        mb = [load_w(d_mb[i], HIDP, 1, f32, f"mb{i + 1}") for i in range(3)]
        nW = [load_w(d_nW[0], NF + MSGD, HIDP, bf16, "nW1"),
              load_w(d_nW[1], HIDP, HIDP, bf16, "nW2"),
              load_w(d_nW[2], HIDP, HIDP, bf16, "nW3"),
              load_w(d_nW[3], HIDP, NF, bf16, "nW4")]
        nb = [load_w(d_nb[i], HIDP, 1, f32, f"nb{i + 1}") for i in range(3)]
        mb4c = wpool.tile([MSGD, 1], f32, tag="mb4c")
        nc.sync.dma_start(mb4c[:, :], d_mb4c[:, :])
        nb4r = wpool.tile([1, NF], bf16, tag="nb4r")
        nc.sync.dma_start(nb4r[:, :], d_nb4r[:, :])

        dstloc = wpool.tile([128, NCHUNKS], f32, tag="dstloc")
        nc.sync.dma_start(dstloc[:, :], d_dstloc[:, :])
        xT = wpool.tile([NF, NP2], bf16, tag="xT")
        nc.sync.dma_start(xT[:, :], d_xT[:, :])
        pmat = wpool.tile([128, NCHK * G], bf16, tag="pmat")
        nc.sync.dma_start(pmat[:, :], d_pmat[:, :])

        iota = wpool.tile([128, NW], f32, tag="iota")
        nc.gpsimd.iota(iota[:, :], pattern=[[1, NW]], base=0,
                       channel_multiplier=0,
                       allow_small_or_imprecise_dtypes=True)
        ones1 = wpool.tile([1, 128], bf16, tag="ones1")
        nc.gpsimd.memset(ones1[:, :], 1.0)

        aggrT = apool.tile([NF, NP2], bf16, tag="aggrT")
        # scatter windows cover cols [0, W*NW); zero the tail
        nc.gpsimd.memset(aggrT[:, W * NW:NP2], 0.0)

        def mlp_front(ksrc_fn, kch, weights, biases, gs):
            """Layers 1-3, weight-stationary over gs supertiles.

            ksrc_fn(layer, prev) -> list of (get_rhs(g) -> AP, kk).
            Returns the layer-3 h tiles."""
            prev = None
            for layer in range(3):
                ksrc = ksrc_fn(layer, prev)
                cur = []
                for m, (m0, mm) in enumerate(HCH):
                    pss = []
                    for g in range(gs):
                        p = mm_psum.tile([128, ST], f32, tag="mmp")
                        pss.append(p)
                    for k, (get_rhs, kk) in enumerate(ksrc):
                        lhs = weights[layer][k][:, m0:m0 + mm]
                        for g in range(gs):
                            nc.tensor.matmul(pss[g][:mm, :], lhs, get_rhs(g),
                                             start=(k == 0),
                                             stop=(k == len(ksrc) - 1))
                    ht = hpool.tile([HCH[m][1], BST * ST], bf16,
                                    tag=f"h{layer}_{m}")
                    for g in range(gs):
                        if layer >= 1:
                            nc.vector.tensor_scalar(
                                ht[:mm, g * ST:(g + 1) * ST], pss[g][:mm, :],
                                biases[layer][m][:mm, :], 0.0,
                                op0=OP.add, op1=OP.max)
                        else:
                            nc.scalar.activation(
                                ht[:mm, g * ST:(g + 1) * ST], pss[g][:mm, :],
                                AF.Relu, bias=biases[layer][m][:mm, :])
                    cur.append(ht)
                prev = cur
            return prev

        # ---- edge phase ----
        acc_state = {}

        def build_sts(blk):
            sts = []
            for i in range(16):
                cidx = blk * 16 + i
                st = spool.tile([128, NW], bf16, tag="st")
                nc.vector.tensor_scalar(
                    st[:, :], iota[:, :], dstloc[:, cidx:cidx + 1], None,
                    op0=OP.is_equal)
                sts.append(st)
            return sts

        def emit_scatter(blk, msgts, sts):
            for i, mt in enumerate(msgts):
                cidx = blk * 16 + i
                w = cidx // C
                cw = cidx % C
                if cw == 0:
                    at = acc_psum.tile([128, NW], f32, tag="acc")
                    acc_state["t"] = at
                nc.tensor.matmul(acc_state["t"][:, :], mt[:, :], sts[i][:, :],
                                 start=(cw == 0), stop=(cw == C - 1),
                                 skip_group_check=True)
                if cw == C - 1:
                    nc.scalar.activation(
                        aggrT[:, w * NW:(w + 1) * NW], acc_state["t"][:, :],
                        AF.Copy)

        def load_block(blk):
            base = blk * (BST * ST)
            tiles = []
            for i, (k0, kk) in enumerate(KIN):
                t = inpool.tile([kk, BST * ST], bf16, tag=f"in{i}")
                nc.sync.dma_start(t[:, :],
                                  d_msg_inT[k0:k0 + kk, base:base + BST * ST])
                tiles.append(t)
            return tiles

        prev_msgts = None
        prev_sts = None
        prev_blk = None
        pending_in = load_block(0)
        for blk in range(NBLK):
            in_t = pending_in
            if blk + 1 < NBLK:
                pending_in = load_block(blk + 1)
            if prev_msgts is not None:
                prev_sts = build_sts(prev_blk)

            def eksrc(layer, prev):
                if layer == 0:
                    return [
                        ((lambda g, t=t, kk=kk: t[:kk, g * ST:(g + 1) * ST]),
                         kk) for t, (k0, kk) in zip(in_t, KIN)]
                return [
                    ((lambda g, h=prev[i], kk=kk: h[:kk, g * ST:(g + 1) * ST]),
                     kk) for i, (k0, kk) in enumerate(HCH)]

            h3 = mlp_front(eksrc, KIN, mW, mb, BST)

            # L4 weight-stationary: out [MSGD, ST] feature-major
            psl4 = []
            for g in range(BST):
                p4 = mm_psum.tile([128, ST], f32, tag="mmp")
                psl4.append(p4)
            for k, (k0, kk) in enumerate(HCH):
                for g in range(BST):
                    nc.tensor.matmul(psl4[g][:, :], mW[3][k][:kk, :],
                                     h3[k][:kk, g * ST:(g + 1) * ST],
                                     start=(k == 0), stop=(k == 2))
            msgts = []
            for g in range(BST):
                mT = mtpool.tile([128, ST], bf16, tag=f"msgT{g}")
                nc.scalar.activation(mT[:, :], psl4[g][:, :], AF.Identity,
                                     bias=mb4c[:, :])
                mt4 = mtpool.tile([128, 4, 128], bf16, tag=f"msgt{g}")
                nc.sync.dma_start_transpose(mt4[:, :, :], mT[:, :])
                for e in range(4):
                    msgts.append(mt4[:, e, :])

            if prev_msgts is not None:
                emit_scatter(prev_blk, prev_msgts, prev_sts)
            prev_msgts, prev_blk = msgts, blk
        prev_sts = build_sts(prev_blk)
        emit_scatter(prev_blk, prev_msgts, prev_sts)

        # ---- node phase ----
        pool_acc = acc_psum.tile([128, NW], f32, tag="acc")
        for t0 in range(0, NT, BST):
            gs = min(BST, NT - t0)

            def nksrc(layer, prev, t0=t0):
                if layer == 0:
                    return [
                        ((lambda g, s=xT, t0=t0:
                          s[:, (t0 + g) * ST:(t0 + g + 1) * ST]), NF),
                        ((lambda g, s=aggrT, t0=t0:
                          s[:, (t0 + g) * ST:(t0 + g + 1) * ST]), MSGD)]
                return [
                    ((lambda g, h=prev[i], kk=kk: h[:kk, g * ST:(g + 1) * ST]),
                     kk) for i, (k0, kk) in enumerate(HCH)]

            h3n = mlp_front(nksrc, None, nW, nb, gs)

            for g in range(gs):
                for e in range(4):
                    tch = (t0 + g) * 4 + e
                    ps = mm_psum.tile([128, ST], f32, tag="mmp")
                    for k, (k0, kk) in enumerate(HCH):
                        nc.tensor.matmul(
                            ps[:, :NF],
                            h3n[k][:kk, g * ST + e * 128:g * ST + (e + 1) * 128],
                            nW[3][k][:kk, :], start=(k == 0), stop=False)
                    nc.tensor.matmul(ps[:, :NF], ones1[:1, :], nb4r[:1, :],
                                     start=False, stop=True)
                    no = mpool.tile([128, NF], bf16, tag="no")
                    nc.scalar.activation(no[:, :], ps[:, :NF], AF.Copy)
                    nc.tensor.matmul(pool_acc[:G, :NF],
                                     pmat[:, tch * G:(tch + 1) * G], no[:, :],
                                     start=(tch == 0), stop=(tch == NCHK - 1),
                                     skip_group_check=True)

        pooled = apool.tile([G, NF], f32, tag="pooled")
        nc.vector.tensor_copy(pooled[:, :], pool_acc[:G, :NF])
        nc.sync.dma_start(d_out[:, :], pooled[:, :])

    nc.compile()
    return nc


def _prep_inputs(x, edge_index, edge_attr, batch, weights, C):
    """Host-side shard/gather/pad. Returns per-core in_maps."""
    E_pad = W * C * 128
    src = np.asarray(edge_index[0], np.int64)
    dst = np.asarray(edge_index[1], np.int64)

    order = np.argsort(dst, kind="stable")
    dsts = dst[order]
    srcs = src[order]

    xT = np.ascontiguousarray(np.asarray(x, np.float32).astype(BF16).T)
    eaT = np.ascontiguousarray(np.asarray(edge_attr, np.float32).astype(BF16).T)
    batch = np.asarray(batch, np.int64)

    bounds = np.searchsorted(dsts, np.arange(0, N_NODES + 1, NPC))

    def pad2(a, r, c):
        out = np.zeros((r, c), a.dtype)
        out[:a.shape[0], :a.shape[1]] = a
        return out

    wcommon = {}
    for i in range(1, 5):
        mw = weights[f"mW{i}"].astype(BF16)
        nw = weights[f"nW{i}"].astype(BF16)
        kr = KINP if i == 1 else HIDP
        kc = MSGD if i == 4 else HIDP
        wcommon[f"mW{i}"] = pad2(mw, kr, kc)
        nr = NF + MSGD if i == 1 else HIDP
        ncol = NF if i == 4 else HIDP
        wcommon[f"nW{i}"] = pad2(nw, nr, ncol)
    for i in range(1, 4):
        wcommon[f"mb{i}"] = pad2(
            weights[f"mb{i}"].reshape(HID, 1).astype(np.float32), HIDP, 1)
        wcommon[f"nb{i}"] = pad2(
            weights[f"nb{i}"].reshape(HID, 1).astype(np.float32), HIDP, 1)
    wcommon["mb4c"] = np.ascontiguousarray(
        weights["mb4"].reshape(MSGD, 1).astype(np.float32))
    wcommon["nb4r"] = np.ascontiguousarray(
        weights["nb4"].reshape(1, NF).astype(BF16))

    garange = np.arange(G)
    in_maps = []
    for k in range(NCORES):
        sl = slice(int(bounds[k]), int(bounds[k + 1]))
        eidx = order[sl]
        dloc = dsts[sl] - k * NPC
        srck = srcs[sl]
        win = dloc // NW
        cnt = np.bincount(win, minlength=W)

        starts = np.repeat(np.arange(W) * C * 128, cnt)
        within = np.arange(len(dloc)) - np.repeat(np.cumsum(cnt) - cnt, cnt)
        pos = starts + within

        msg_inT = np.zeros((KINP, E_pad), BF16)
        msg_inT[0:NF, pos] = xT[:, k * NPC + dloc]
        msg_inT[NF:2 * NF, pos] = xT[:, srck]
        msg_inT[2 * NF:2 * NF + EF, pos] = eaT[:, eidx]

        dl = np.full(E_pad, -1.0, np.float32)
        dl[pos] = (dloc - win * NW).astype(np.float32)
        dstloc = np.ascontiguousarray(dl.reshape(E_pad // 128, 128).T)

        xTn = np.zeros((NF, NP2), BF16)
        xTn[:, :NPC] = xT[:, k * NPC:(k + 1) * NPC]

        bl = np.full(NP2, -1, np.int64)
        bl[:NPC] = batch[k * NPC:(k + 1) * NPC]
        P = (bl[:, None] == garange[None, :]).astype(BF16)
        pmat = np.ascontiguousarray(
            P.reshape(NCHK, 128, G).transpose(1, 0, 2).reshape(128, NCHK * G))

        in_map = dict(wcommon)
        in_map.update(msg_inT=msg_inT, dstloc=dstloc, xT=xTn, pmat=pmat)
        in_maps.append(in_map)
    return in_maps


def kernel(**inputs):
    global LAST_EXEC_NS
    from concourse.bass_utils import run_bass_kernel_spmd

    x = np.asarray(inputs["x"], np.float32)
    edge_index = np.asarray(inputs["edge_index"])
    edge_attr = np.asarray(inputs["edge_attr"], np.float32)
    batch = np.asarray(inputs["batch"])

    # chunk count per window from the actual data (uniform across cores)
    dst = np.asarray(edge_index[1], np.int64)
    dloc_all = dst % NPC
    core_all = dst // NPC
    win_all = dloc_all // NW
    cnt = np.bincount(core_all * W + win_all, minlength=NCORES * W)
    C = int(np.ceil(cnt.max() / 128.0))
    C = max(C, 8)
    while (W * C) % 16 != 0:
        C += 1

    key = C
    if key not in _BUILD_CACHE:
        _BUILD_CACHE[key] = _build_nc(C)
    nc = _BUILD_CACHE[key]

    in_maps = _prep_inputs(x, edge_index, edge_attr, batch, inputs, C)

    res = run_bass_kernel_spmd(nc, in_maps, list(range(NCORES)), trace=TRACE)
    LAST_EXEC_NS = res.exec_time_ns

    total = np.zeros((G, NF), np.float64)
    for r in res.results:
        total += np.asarray(r["partial"], np.float64)

    counts = np.bincount(np.asarray(batch, np.int64), minlength=G)
    pooled = (total / np.maximum(counts, 1)[:, None]).astype(np.float32)
    out = pooled @ np.asarray(inputs["linW"], np.float32) + np.asarray(
        inputs["linb"], np.float32)
    return out.astype(np.float32)


# revision 27
# speedup vs baseline: 2.0876x; 1.1820x over previous
"""GNN message-passing + pooling kernel for 8 Trainium2 NeuronCores.

Strategy (per the sharding hint):
  - Host: sort edges by dst, partition the 50k nodes into 8 contiguous
    ranges of 6250; each core gets the edges targeting its node range
    (disjoint scatter -> no cross-core reduction needed).
  - Host gathers x[dst], x[src], edge_attr into a transposed bf16
    [320, E_pad] tensor per core (edges grouped into 481-node scatter
    windows, padded to a uniform chunk count so the device program is
    identical across cores).
  - Device (per core): 4-layer message MLP in transposed-activation
    layout processed in 2048-edge blocks (4x512 supertiles).  Each
    weight chunk is kept stationary on the PE array for 4 consecutive
    matmuls (amortizes LDWEIGHTS, which otherwise serializes ~100ns per
    matmul).  Layer 4 is computed weight-stationary into a feature-major
    [msg_dim, edges] PSUM tile, bias fused into the PSUM->SBUF copy on
    the scalar engine, then DMA-transposed (xbar) into edge-major
    [128, 128] chunks for the scatter.  Scatter-add via one-hot matmuls
    (one-hot built on DVE with iota + is_equal against per-edge local
    dst), deferred by one block so the transposes are off the critical
    path.  Node MLP over the core's 6250 nodes with the same blocked
    structure, per-graph sum-pooling accumulated in a single PSUM bank.
    Output: [32, 128] partial per-graph sums.
  - Host: sum the 8 partials, divide by per-graph node counts, apply the
    final [128, 16] linear.
"""

import sys

if "/opt/trn_rl_repo" not in sys.path:
    sys.path.insert(0, "/opt/trn_rl_repo")

import numpy as np
import ml_dtypes

BF16 = ml_dtypes.bfloat16

# Problem dims
N_NODES = 50000
N_EDGES = 800000
NF = 128          # node feature dim
EF = 64           # edge feature dim
MSGD = 128        # message dim
HID = 300         # MLP hidden
G = 32            # graphs
NCORES = 8

# Tiling config
NPC = N_NODES // NCORES   # 6250 nodes per core
NW = 241                  # nodes per scatter window
W = 26                    # windows per core (26*241 = 6266 >= 6250)
ST = 512                  # edge supertile (free dim per matmul)
BST = 4                   # supertiles per block (weight-stationary reuse)
NP2 = 6656                # padded nodes per core for node MLP (13*512)
NT = NP2 // ST            # node supertiles
NCHK = NP2 // 128         # node chunks for pooling
HIDP = 384                # HID zero-padded to full 128-row K chunks
KINP = 384                # 2*NF+EF zero-padded likewise

TRACE = False             # set True from test harness to profile core 0
LAST_EXEC_NS = None

_BUILD_CACHE = {}


def _chunks(total, step=128):
    return [(o, min(step, total - o)) for o in range(0, total, step)]


def _build_nc(C):
    """Build the (single) SPMD Bass program. C = 128-edge chunks per window
    (multiple of 16 so each window is a whole number of 2048-edge blocks)."""
    import concourse.bacc as bacc
    import concourse.tile as tile
    from concourse import mybir
    from contextlib import ExitStack

    f32 = mybir.dt.float32
    bf16 = mybir.dt.bfloat16
    AF = mybir.ActivationFunctionType
    OP = mybir.AluOpType

    E_pad = W * C * 128
    NCHUNKS = W * C
    NBLK = NCHUNKS // 16      # 2048-edge blocks

    nc = bacc.Bacc("TRN2", target_bir_lowering=False, debug=False,
                   num_devices=NCORES)

    # --- DRAM I/O ---
    d_msg_inT = nc.dram_tensor("msg_inT", [KINP, E_pad], bf16,
                               kind="ExternalInput")
    d_dstloc = nc.dram_tensor("dstloc", [128, NCHUNKS], f32,
                              kind="ExternalInput")
    d_xT = nc.dram_tensor("xT", [NF, NP2], bf16, kind="ExternalInput")
    d_pmat = nc.dram_tensor("pmat", [128, NCHK * G], bf16,
                            kind="ExternalInput")
    fp8 = mybir.dt.float8e4

    d_mW1 = nc.dram_tensor("mW1", [KINP, HIDP], bf16, kind="ExternalInput")
    d_mW4 = nc.dram_tensor("mW4", [HIDP, MSGD], bf16, kind="ExternalInput")
    d_nW1 = nc.dram_tensor("nW1", [NF + MSGD, HIDP], bf16,
                           kind="ExternalInput")
    d_nW4 = nc.dram_tensor("nW4", [HIDP, NF], bf16, kind="ExternalInput")
    # L2/L3 weights: fp8 DoubleRow pair (K rows 0..255, x8 scale) + bf16
    # remainder (K rows 256..383, x8 scale)
    d_wdr = {}
    d_wrem = {}
    for nm in ("mW2", "mW3", "nW2", "nW3"):
        d_wdr[nm] = nc.dram_tensor(f"{nm}dr", [128, 2 * HIDP], fp8,
                                   kind="ExternalInput")
        d_wrem[nm] = nc.dram_tensor(f"{nm}rem", [128, HIDP], bf16,
                                    kind="ExternalInput")
    d_mb = [nc.dram_tensor(f"mb{i}", [HIDP, 1], f32, kind="ExternalInput")
            for i in range(1, 4)]
    d_mb4c = nc.dram_tensor("mb4c", [MSGD, 1], f32, kind="ExternalInput")
    d_nb = [nc.dram_tensor(f"nb{i}", [HIDP, 1], f32, kind="ExternalInput")
            for i in range(1, 4)]
    d_nb4r = nc.dram_tensor("nb4r", [1, NF], bf16, kind="ExternalInput")
    d_out = nc.dram_tensor("partial", [G, NF], f32, kind="ExternalOutput")

    HCH = _chunks(HIDP)         # [(0,128),(128,128),(256,128)]
    KIN = _chunks(KINP)         # [(0,128),(128,128),(256,128)]

    with tile.TileContext(nc) as tc, ExitStack() as ctx:
        wpool = ctx.enter_context(tc.tile_pool(name="w", bufs=1))
        apool = ctx.enter_context(tc.tile_pool(name="agg", bufs=1))
        inpool = ctx.enter_context(tc.tile_pool(name="in", bufs=3))
        hpool = ctx.enter_context(tc.tile_pool(name="h", bufs=2))
        mtpool = ctx.enter_context(tc.tile_pool(name="mt", bufs=2))
        mpool = ctx.enter_context(tc.tile_pool(name="m", bufs=4))
        spool = ctx.enter_context(tc.tile_pool(name="s", bufs=20))
        mm_psum = ctx.enter_context(
            tc.tile_pool(name="mmp", bufs=7, space="PSUM"))
        acc_psum = ctx.enter_context(
            tc.tile_pool(name="accp", bufs=1, space="PSUM"))

        def load_w(dram, K, N, dt, name):
            tiles = []
            for i, (k0, kk) in enumerate(_chunks(K)):
                t = wpool.tile([kk, N], dt, tag=f"{name}{i}")
                nc.sync.dma_start(t[:, :], dram[k0:k0 + kk, :])
                tiles.append(t)
            return tiles

        mW1 = load_w(d_mW1, KINP, HIDP, bf16, "mW1")
        mW4 = load_w(d_mW4, HIDP, MSGD, bf16, "mW4")
        nW1 = load_w(d_nW1, NF + MSGD, HIDP, bf16, "nW1")
        nW4 = load_w(d_nW4, HIDP, NF, bf16, "nW4")
        wdr = {}
        wrem = {}
        for nm in ("mW2", "mW3", "nW2", "nW3"):
            t3 = wpool.tile([128, 2, HIDP], fp8, tag=f"{nm}dr")
            nc.sync.dma_start(t3[:, :, :], d_wdr[nm][:, :])
            wdr[nm] = t3
            tr = wpool.tile([128, HIDP], bf16, tag=f"{nm}rem")
            nc.sync.dma_start(tr[:, :], d_wrem[nm][:, :])
            wrem[nm] = tr
        mb = [load_w(d_mb[i], HIDP, 1, f32, f"mb{i + 1}") for i in range(3)]
        nb = [load_w(d_nb[i], HIDP, 1, f32, f"nb{i + 1}") for i in range(3)]
        mb4c = wpool.tile([MSGD, 1], f32, tag="mb4c")
        nc.sync.dma_start(mb4c[:, :], d_mb4c[:, :])
        nb4r = wpool.tile([1, NF], bf16, tag="nb4r")
        nc.sync.dma_start(nb4r[:, :], d_nb4r[:, :])

        dstloc = wpool.tile([128, NCHUNKS], f32, tag="dstloc")
        nc.sync.dma_start(dstloc[:, :], d_dstloc[:, :])
        xT = wpool.tile([NF, NP2], bf16, tag="xT")
        nc.sync.dma_start(xT[:, :], d_xT[:, :])
        pmat = wpool.tile([128, NCHK * G], bf16, tag="pmat")
        nc.sync.dma_start(pmat[:, :], d_pmat[:, :])

        iota = wpool.tile([128, NW], f32, tag="iota")
        nc.gpsimd.iota(iota[:, :], pattern=[[1, NW]], base=0,
                       channel_multiplier=0,
                       allow_small_or_imprecise_dtypes=True)
        ones1 = wpool.tile([1, 128], bf16, tag="ones1")
        nc.gpsimd.memset(ones1[:, :], 1.0)

        aggrT = apool.tile([NF, NP2], bf16, tag="aggrT")
        # scatter windows cover cols [0, W*NW); zero the tail
        nc.gpsimd.memset(aggrT[:, W * NW:NP2], 0.0)

        DR = mybir.MatmulPerfMode.DoubleRow

        def mlp_front(l1_ksrc, W1ch, w2d, w2r, w3d, w3r, biases, gs):
            """Layers 1-3, weight-stationary over gs supertiles.
            L1 bf16; L2/L3 fp8 DoubleRow (K 0..255) + bf16 remainder, with
            the x8 weight scale divided out in the consuming activation.
            Returns the layer-3 h chunk tiles."""
            # ---- L1 (bf16) -> h1 fp8 dr-pair + bf16 remainder ----
            hd1 = hpool.tile([128, 2, BST * ST], fp8, tag="hd0")
            hr1 = hpool.tile([128, BST * ST], bf16, tag="hr0")
            for m, (m0, mm) in enumerate(HCH):
                pss = []
                for g in range(gs):
                    p = mm_psum.tile([128, ST], f32, tag="mmp")
                    pss.append(p)
                for k, (get_rhs, kk) in enumerate(l1_ksrc):
                    lhs = W1ch[k][:, m0:m0 + mm]
                    for g in range(gs):
                        nc.tensor.matmul(pss[g][:mm, :], lhs, get_rhs(g),
                                         start=(k == 0),
                                         stop=(k == len(l1_ksrc) - 1))
                for g in range(gs):
                    dst = (hd1[:, m, g * ST:(g + 1) * ST] if m < 2
                           else hr1[:, g * ST:(g + 1) * ST])
                    nc.vector.tensor_scalar(
                        dst, pss[g][:mm, :], biases[0][m][:mm, :], 0.0,
                        op0=OP.add, op1=OP.max)

            # ---- L2 (fp8 DR) -> h2 fp8 dr-pair + bf16 remainder ----
            hd2 = hpool.tile([128, 2, BST * ST], fp8, tag="hd1")
            hr2 = hpool.tile([128, BST * ST], bf16, tag="hr1")
            for m, (m0, mm) in enumerate(HCH):
                pss = []
                for g in range(gs):
                    p = mm_psum.tile([128, ST], f32, tag="mmp")
                    pss.append(p)
                for g in range(gs):
                    nc.tensor.matmul(pss[g][:mm, :], w2d[:, :, m0:m0 + mm],
                                     hd1[:, :, g * ST:(g + 1) * ST],
                                     start=True, stop=False, perf_mode=DR)
                for g in range(gs):
                    nc.tensor.matmul(pss[g][:mm, :], w2r[:, m0:m0 + mm],
                                     hr1[:, g * ST:(g + 1) * ST],
                                     start=False, stop=True)
                for g in range(gs):
                    dst = (hd2[:, m, g * ST:(g + 1) * ST] if m < 2
                           else hr2[:, g * ST:(g + 1) * ST])
                    nc.scalar.activation(dst, pss[g][:mm, :], AF.Relu,
                                         bias=biases[1][m][:mm, :],
                                         scale=0.125)

            # ---- L3 (fp8 DR) -> h3 bf16 chunks ----
            cur = []
            for m, (m0, mm) in enumerate(HCH):
                pss = []
                for g in range(gs):
                    p = mm_psum.tile([128, ST], f32, tag="mmp")
                    pss.append(p)
                for g in range(gs):
                    nc.tensor.matmul(pss[g][:mm, :], w3d[:, :, m0:m0 + mm],
                                     hd2[:, :, g * ST:(g + 1) * ST],
                                     start=True, stop=False, perf_mode=DR)
                for g in range(gs):
                    nc.tensor.matmul(pss[g][:mm, :], w3r[:, m0:m0 + mm],
                                     hr2[:, g * ST:(g + 1) * ST],
                                     start=False, stop=True)
                ht = hpool.tile([128, BST * ST], bf16, tag=f"h2_{m}")
                for g in range(gs):
                    nc.scalar.activation(ht[:mm, g * ST:(g + 1) * ST],
                                         pss[g][:mm, :], AF.Relu,
                                         bias=biases[2][m][:mm, :],
                                         scale=0.125)
                cur.append(ht)
            return cur

        # ---- edge phase ----
        acc_state = {}

        def build_sts(blk):
            sts = []
            for i in range(16):
                cidx = blk * 16 + i
                st = spool.tile([128, NW], bf16, tag="st")
                nc.vector.tensor_scalar(
                    st[:, :], iota[:, :], dstloc[:, cidx:cidx + 1], None,
                    op0=OP.is_equal)
                sts.append(st)
            return sts

        def emit_scatter(blk, msgts, sts):
            for i, mt in enumerate(msgts):
                cidx = blk * 16 + i
                w = cidx // C
                cw = cidx % C
                if cw == 0:
                    at = acc_psum.tile([128, NW], f32, tag="acc")
                    acc_state["t"] = at
                nc.tensor.matmul(acc_state["t"][:, :], mt[:, :], sts[i][:, :],
                                 start=(cw == 0), stop=(cw == C - 1),
                                 skip_group_check=True)
                if cw == C - 1:
                    nc.scalar.activation(
                        aggrT[:, w * NW:(w + 1) * NW], acc_state["t"][:, :],
                        AF.Copy)

        def load_block(blk):
            base = blk * (BST * ST)
            tiles = []
            for i, (k0, kk) in enumerate(KIN):
                t = inpool.tile([kk, BST * ST], bf16, tag=f"in{i}")
                nc.sync.dma_start(t[:, :],
                                  d_msg_inT[k0:k0 + kk, base:base + BST * ST])
                tiles.append(t)
            return tiles

        prev_msgts = None
        prev_sts = None
        prev_blk = None
        pending_in = load_block(0)
        for blk in range(NBLK):
            in_t = pending_in
            if blk + 1 < NBLK:
                pending_in = load_block(blk + 1)
            if prev_msgts is not None:
                prev_sts = build_sts(prev_blk)

            eksrc = [
                ((lambda g, t=t, kk=kk: t[:kk, g * ST:(g + 1) * ST]), kk)
                for t, (k0, kk) in zip(in_t, KIN)]

            h3 = mlp_front(eksrc, mW1, wdr["mW2"], wrem["mW2"],
                           wdr["mW3"], wrem["mW3"], mb, BST)

            # L4 weight-stationary: out [MSGD, ST] feature-major
            psl4 = []
            for g in range(BST):
                p4 = mm_psum.tile([128, ST], f32, tag="mmp")
                psl4.append(p4)
            for k, (k0, kk) in enumerate(HCH):
                for g in range(BST):
                    nc.tensor.matmul(psl4[g][:, :], mW4[k][:kk, :],
                                     h3[k][:kk, g * ST:(g + 1) * ST],
                                     start=(k == 0), stop=(k == 2))
            msgts = []
            for g in range(BST):
                mT = mtpool.tile([128, ST], bf16, tag=f"msgT{g}")
                nc.scalar.activation(mT[:, :], psl4[g][:, :], AF.Identity,
                                     bias=mb4c[:, :])
                mt4 = mtpool.tile([128, 4, 128], bf16, tag=f"msgt{g}")
                nc.sync.dma_start_transpose(mt4[:, :, :], mT[:, :])
                for e in range(4):
                    msgts.append(mt4[:, e, :])

            if prev_msgts is not None:
                emit_scatter(prev_blk, prev_msgts, prev_sts)
            prev_msgts, prev_blk = msgts, blk
        prev_sts = build_sts(prev_blk)
        emit_scatter(prev_blk, prev_msgts, prev_sts)

        # ---- node phase ----
        pool_acc = acc_psum.tile([128, NW], f32, tag="acc")
        for t0 in range(0, NT, BST):
            gs = min(BST, NT - t0)

            nksrc = [
                ((lambda g, s=xT, t0=t0:
                  s[:, (t0 + g) * ST:(t0 + g + 1) * ST]), NF),
                ((lambda g, s=aggrT, t0=t0:
                  s[:, (t0 + g) * ST:(t0 + g + 1) * ST]), MSGD)]

            h3n = mlp_front(nksrc, nW1, wdr["nW2"], wrem["nW2"],
                            wdr["nW3"], wrem["nW3"], nb, gs)

            for g in range(gs):
                for e in range(4):
                    tch = (t0 + g) * 4 + e
                    ps = mm_psum.tile([128, ST], f32, tag="mmp")
                    for k, (k0, kk) in enumerate(HCH):
                        nc.tensor.matmul(
                            ps[:, :NF],
                            h3n[k][:kk, g * ST + e * 128:g * ST + (e + 1) * 128],
                            nW4[k][:kk, :], start=(k == 0), stop=False)
                    nc.tensor.matmul(ps[:, :NF], ones1[:1, :], nb4r[:1, :],
                                     start=False, stop=True)
                    no = mpool.tile([128, NF], bf16, tag="no")
                    nc.scalar.activation(no[:, :], ps[:, :NF], AF.Copy)
                    nc.tensor.matmul(pool_acc[:G, :NF],
                                     pmat[:, tch * G:(tch + 1) * G], no[:, :],
                                     start=(tch == 0), stop=(tch == NCHK - 1),
                                     skip_group_check=True)

        pooled = apool.tile([G, NF], f32, tag="pooled")
        nc.vector.tensor_copy(pooled[:, :], pool_acc[:G, :NF])
        nc.sync.dma_start(d_out[:, :], pooled[:, :])

    nc.compile()
    return nc


def _prep_inputs(x, edge_index, edge_attr, batch, weights, C):
    """Host-side shard/gather/pad. Returns per-core in_maps."""
    E_pad = W * C * 128
    src = np.asarray(edge_index[0], np.int64)
    dst = np.asarray(edge_index[1], np.int64)

    order = np.argsort(dst, kind="stable")
    dsts = dst[order]
    srcs = src[order]

    xT = np.ascontiguousarray(np.asarray(x, np.float32).astype(BF16).T)
    eaT = np.ascontiguousarray(np.asarray(edge_attr, np.float32).astype(BF16).T)
    batch = np.asarray(batch, np.int64)

    bounds = np.searchsorted(dsts, np.arange(0, N_NODES + 1, NPC))

    def pad2(a, r, c):
        out = np.zeros((r, c), a.dtype)
        out[:a.shape[0], :a.shape[1]] = a
        return out

    FP8 = ml_dtypes.float8_e4m3fn
    wcommon = {}
    wcommon["mW1"] = pad2(weights["mW1"].astype(BF16), KINP, HIDP)
    wcommon["mW4"] = pad2(weights["mW4"].astype(BF16), HIDP, MSGD)
    wcommon["nW1"] = pad2(weights["nW1"].astype(BF16), NF + MSGD, HIDP)
    wcommon["nW4"] = pad2(weights["nW4"].astype(BF16), HIDP, NF)
    for nm in ("mW2", "mW3", "nW2", "nW3"):
        wp = pad2(weights[nm].astype(np.float32), HIDP, HIDP) * 8.0
        dr = wp[:256].reshape(2, 128, HIDP).transpose(1, 0, 2)
        wcommon[f"{nm}dr"] = np.ascontiguousarray(
            dr.reshape(128, 2 * HIDP).astype(FP8))
        wcommon[f"{nm}rem"] = np.ascontiguousarray(wp[256:HIDP].astype(BF16))
    for i in range(1, 4):
        wcommon[f"mb{i}"] = pad2(
            weights[f"mb{i}"].reshape(HID, 1).astype(np.float32), HIDP, 1)
        wcommon[f"nb{i}"] = pad2(
            weights[f"nb{i}"].reshape(HID, 1).astype(np.float32), HIDP, 1)
    wcommon["mb4c"] = np.ascontiguousarray(
        weights["mb4"].reshape(MSGD, 1).astype(np.float32))
    wcommon["nb4r"] = np.ascontiguousarray(
        weights["nb4"].reshape(1, NF).astype(BF16))

    garange = np.arange(G)
    in_maps = []
    for k in range(NCORES):
        sl = slice(int(bounds[k]), int(bounds[k + 1]))
        eidx = order[sl]
        dloc = dsts[sl] - k * NPC
        srck = srcs[sl]
        win = dloc // NW
        cnt = np.bincount(win, minlength=W)

        starts = np.repeat(np.arange(W) * C * 128, cnt)
        within = np.arange(len(dloc)) - np.repeat(np.cumsum(cnt) - cnt, cnt)
        pos = starts + within

        msg_inT = np.zeros((KINP, E_pad), BF16)
        msg_inT[0:NF, pos] = xT[:, k * NPC + dloc]
        msg_inT[NF:2 * NF, pos] = xT[:, srck]
        msg_inT[2 * NF:2 * NF + EF, pos] = eaT[:, eidx]

        dl = np.full(E_pad, -1.0, np.float32)
        dl[pos] = (dloc - win * NW).astype(np.float32)
        dstloc = np.ascontiguousarray(dl.reshape(E_pad // 128, 128).T)

        xTn = np.zeros((NF, NP2), BF16)
        xTn[:, :NPC] = xT[:, k * NPC:(k + 1) * NPC]

        bl = np.full(NP2, -1, np.int64)
        bl[:NPC] = batch[k * NPC:(k + 1) * NPC]
        P = (bl[:, None] == garange[None, :]).astype(BF16)
        pmat = np.ascontiguousarray(
            P.reshape(NCHK, 128, G).transpose(1, 0, 2).reshape(128, NCHK * G))

        in_map = dict(wcommon)
        in_map.update(msg_inT=msg_inT, dstloc=dstloc, xT=xTn, pmat=pmat)
        in_maps.append(in_map)
    return in_maps


def kernel(**inputs):
    global LAST_EXEC_NS
    from concourse.bass_utils import run_bass_kernel_spmd

    x = np.asarray(inputs["x"], np.float32)
    edge_index = np.asarray(inputs["edge_index"])
    edge_attr = np.asarray(inputs["edge_attr"], np.float32)
    batch = np.asarray(inputs["batch"])

    # chunk count per window from the actual data (uniform across cores)
    dst = np.asarray(edge_index[1], np.int64)
    dloc_all = dst % NPC
    core_all = dst // NPC
    win_all = dloc_all // NW
    cnt = np.bincount(core_all * W + win_all, minlength=NCORES * W)
    C = int(np.ceil(cnt.max() / 128.0))
    C = max(C, 8)
    while (W * C) % 16 != 0:
        C += 1

    key = C
    if key not in _BUILD_CACHE:
        _BUILD_CACHE[key] = _build_nc(C)
    nc = _BUILD_CACHE[key]

    in_maps = _prep_inputs(x, edge_index, edge_attr, batch, inputs, C)

    res = run_bass_kernel_spmd(nc, in_maps, list(range(NCORES)), trace=TRACE)
    LAST_EXEC_NS = res.exec_time_ns

    total = np.zeros((G, NF), np.float64)
    for r in res.results:
        total += np.asarray(r["partial"], np.float64)

    counts = np.bincount(np.asarray(batch, np.int64), minlength=G)
    pooled = (total / np.maximum(counts, 1)[:, None]).astype(np.float32)
    out = pooled @ np.asarray(inputs["linW"], np.float32) + np.asarray(
        inputs["linb"], np.float32)
    return out.astype(np.float32)


# revision 29
# speedup vs baseline: 2.1165x; 1.0139x over previous
"""GNN message-passing + pooling kernel for 8 Trainium2 NeuronCores.

Strategy (per the sharding hint):
  - Host: sort edges by dst, partition the 50k nodes into 8 contiguous
    ranges of 6250; each core gets the edges targeting its node range
    (disjoint scatter -> no cross-core reduction needed).
  - Host gathers x[dst], x[src], edge_attr into a transposed bf16
    [320, E_pad] tensor per core (edges grouped into 481-node scatter
    windows, padded to a uniform chunk count so the device program is
    identical across cores).
  - Device (per core): 4-layer message MLP in transposed-activation
    layout processed in 2048-edge blocks (4x512 supertiles).  Each
    weight chunk is kept stationary on the PE array for 4 consecutive
    matmuls (amortizes LDWEIGHTS, which otherwise serializes ~100ns per
    matmul).  Layer 4 is computed weight-stationary into a feature-major
    [msg_dim, edges] PSUM tile, bias fused into the PSUM->SBUF copy on
    the scalar engine, then DMA-transposed (xbar) into edge-major
    [128, 128] chunks for the scatter.  Scatter-add via one-hot matmuls
    (one-hot built on DVE with iota + is_equal against per-edge local
    dst), deferred by one block so the transposes are off the critical
    path.  Node MLP over the core's 6250 nodes with the same blocked
    structure, per-graph sum-pooling accumulated in a single PSUM bank.
    Output: [32, 128] partial per-graph sums.
  - Host: sum the 8 partials, divide by per-graph node counts, apply the
    final [128, 16] linear.
"""

import sys

if "/opt/trn_rl_repo" not in sys.path:
    sys.path.insert(0, "/opt/trn_rl_repo")

import numpy as np
import ml_dtypes

BF16 = ml_dtypes.bfloat16

# Problem dims
N_NODES = 50000
N_EDGES = 800000
NF = 128          # node feature dim
EF = 64           # edge feature dim
MSGD = 128        # message dim
HID = 300         # MLP hidden
G = 32            # graphs
NCORES = 8

# Tiling config
NPC = N_NODES // NCORES   # 6250 nodes per core
NW = 241                  # nodes per scatter window
W = 26                    # windows per core (26*241 = 6266 >= 6250)
ST = 512                  # edge supertile (free dim per matmul)
BST = 4                   # supertiles per block (weight-stationary reuse)
NP2 = 6656                # padded nodes per core for node MLP (13*512)
NT = NP2 // ST            # node supertiles
NCHK = NP2 // 128         # node chunks for pooling
HIDP = 384                # HID zero-padded to full 128-row K chunks
KINP = 384                # 2*NF+EF zero-padded likewise

TRACE = False             # set True from test harness to profile core 0
LAST_EXEC_NS = None

_BUILD_CACHE = {}


def _chunks(total, step=128):
    return [(o, min(step, total - o)) for o in range(0, total, step)]


def _build_nc(C):
    """Build the (single) SPMD Bass program. C = 128-edge chunks per window
    (multiple of 16 so each window is a whole number of 2048-edge blocks)."""
    import concourse.bacc as bacc
    import concourse.tile as tile
    from concourse import mybir
    from contextlib import ExitStack

    f32 = mybir.dt.float32
    bf16 = mybir.dt.bfloat16
    AF = mybir.ActivationFunctionType
    OP = mybir.AluOpType

    E_pad = W * C * 128
    NCHUNKS = W * C
    NBLK = NCHUNKS // 16      # 2048-edge blocks

    nc = bacc.Bacc("TRN2", target_bir_lowering=False, debug=False,
                   num_devices=NCORES)

    # --- DRAM I/O ---
    d_msg_inT = nc.dram_tensor("msg_inT", [KINP, E_pad], bf16,
                               kind="ExternalInput")
    d_dstloc = nc.dram_tensor("dstloc", [128, NCHUNKS], f32,
                              kind="ExternalInput")
    d_xT = nc.dram_tensor("xT", [NF, NP2], bf16, kind="ExternalInput")
    d_pmat = nc.dram_tensor("pmat", [128, NCHK * G], bf16,
                            kind="ExternalInput")
    fp8 = mybir.dt.float8e4

    d_mW1 = nc.dram_tensor("mW1", [KINP, HIDP], bf16, kind="ExternalInput")
    d_mW4 = nc.dram_tensor("mW4", [HIDP, MSGD], bf16, kind="ExternalInput")
    d_nW1 = nc.dram_tensor("nW1", [NF + MSGD, HIDP], bf16,
                           kind="ExternalInput")
    d_nW4 = nc.dram_tensor("nW4", [HIDP, NF], bf16, kind="ExternalInput")
    # L2/L3 weights: fp8 DoubleRow pair (K rows 0..255, x8 scale) + bf16
    # remainder (K rows 256..383, x8 scale)
    d_wdr = {}
    d_wrem = {}
    for nm in ("mW2", "mW3", "nW2", "nW3"):
        d_wdr[nm] = nc.dram_tensor(f"{nm}dr", [128, 2 * HIDP], fp8,
                                   kind="ExternalInput")
        d_wrem[nm] = nc.dram_tensor(f"{nm}rem", [128, HIDP], bf16,
                                    kind="ExternalInput")
    d_mb = [nc.dram_tensor(f"mb{i}", [HIDP, 1], f32, kind="ExternalInput")
            for i in range(1, 4)]
    d_mb4c = nc.dram_tensor("mb4c", [MSGD, 1], f32, kind="ExternalInput")
    d_nb = [nc.dram_tensor(f"nb{i}", [HIDP, 1], f32, kind="ExternalInput")
            for i in range(1, 4)]
    d_nb4r = nc.dram_tensor("nb4r", [1, NF], bf16, kind="ExternalInput")
    d_out = nc.dram_tensor("partial", [G, NF], f32, kind="ExternalOutput")

    HCH = _chunks(HIDP)         # [(0,128),(128,128),(256,128)]
    KIN = _chunks(KINP)         # [(0,128),(128,128),(256,128)]

    with tile.TileContext(nc) as tc, ExitStack() as ctx:
        wpool = ctx.enter_context(tc.tile_pool(name="w", bufs=1))
        apool = ctx.enter_context(tc.tile_pool(name="agg", bufs=1))
        inpool = ctx.enter_context(tc.tile_pool(name="in", bufs=3))
        hpool = ctx.enter_context(tc.tile_pool(name="h", bufs=2))
        mtpool = ctx.enter_context(tc.tile_pool(name="mt", bufs=2))
        mpool = ctx.enter_context(tc.tile_pool(name="m", bufs=4))
        spool = ctx.enter_context(tc.tile_pool(name="s", bufs=20))
        mm_psum = ctx.enter_context(
            tc.tile_pool(name="mmp", bufs=7, space="PSUM"))
        acc_psum = ctx.enter_context(
            tc.tile_pool(name="accp", bufs=1, space="PSUM"))

        def load_w(dram, K, N, dt, name):
            tiles = []
            for i, (k0, kk) in enumerate(_chunks(K)):
                t = wpool.tile([kk, N], dt, tag=f"{name}{i}")
                nc.sync.dma_start(t[:, :], dram[k0:k0 + kk, :])
                tiles.append(t)
            return tiles

        def load_block(blk):
            base = blk * (BST * ST)
            tiles = []
            for i, (k0, kk) in enumerate(KIN):
                t = inpool.tile([kk, BST * ST], bf16, tag=f"in{i}")
                nc.sync.dma_start(t[:, :],
                                  d_msg_inT[k0:k0 + kk, base:base + BST * ST])
                tiles.append(t)
            return tiles

        # first block's inputs + L1 weights first so the PE can start early;
        # the rest of the (large) resident loads follow on the same queue
        pending_in = load_block(0)
        mW1 = load_w(d_mW1, KINP, HIDP, bf16, "mW1")
        mW4 = load_w(d_mW4, HIDP, MSGD, bf16, "mW4")
        nW1 = load_w(d_nW1, NF + MSGD, HIDP, bf16, "nW1")
        nW4 = load_w(d_nW4, HIDP, NF, bf16, "nW4")
        wdr = {}
        wrem = {}
        for nm in ("mW2", "mW3", "nW2", "nW3"):
            t3 = wpool.tile([128, 2, HIDP], fp8, tag=f"{nm}dr")
            nc.sync.dma_start(t3[:, :, :], d_wdr[nm][:, :])
            wdr[nm] = t3
            tr = wpool.tile([128, HIDP], bf16, tag=f"{nm}rem")
            nc.sync.dma_start(tr[:, :], d_wrem[nm][:, :])
            wrem[nm] = tr
        mb = [load_w(d_mb[i], HIDP, 1, f32, f"mb{i + 1}") for i in range(3)]
        nb = [load_w(d_nb[i], HIDP, 1, f32, f"nb{i + 1}") for i in range(3)]
        mb4c = wpool.tile([MSGD, 1], f32, tag="mb4c")
        nc.sync.dma_start(mb4c[:, :], d_mb4c[:, :])
        nb4r = wpool.tile([1, NF], bf16, tag="nb4r")
        nc.sync.dma_start(nb4r[:, :], d_nb4r[:, :])

        dstloc = wpool.tile([128, NCHUNKS], f32, tag="dstloc")
        nc.sync.dma_start(dstloc[:, :], d_dstloc[:, :])
        xT = wpool.tile([NF, NP2], bf16, tag="xT")
        nc.sync.dma_start(xT[:, :], d_xT[:, :])
        pmat = wpool.tile([128, NCHK * G], bf16, tag="pmat")
        nc.sync.dma_start(pmat[:, :], d_pmat[:, :])

        iota = wpool.tile([128, NW], f32, tag="iota")
        nc.gpsimd.iota(iota[:, :], pattern=[[1, NW]], base=0,
                       channel_multiplier=0,
                       allow_small_or_imprecise_dtypes=True)
        ones1 = wpool.tile([1, 128], bf16, tag="ones1")
        nc.gpsimd.memset(ones1[:, :], 1.0)

        aggrT = apool.tile([NF, NP2], bf16, tag="aggrT")
        # scatter windows cover cols [0, W*NW); zero the tail
        nc.gpsimd.memset(aggrT[:, W * NW:NP2], 0.0)

        DR = mybir.MatmulPerfMode.DoubleRow

        def mlp_front(l1_ksrc, W1ch, w2d, w2r, w3d, w3r, biases, gs):
            """Layers 1-3, weight-stationary over gs supertiles.
            L1 bf16; L2/L3 fp8 DoubleRow (K 0..255) + bf16 remainder, with
            the x8 weight scale divided out in the consuming activation.
            Returns the layer-3 h chunk tiles."""
            # ---- L1 (bf16) -> h1 fp8 dr-pair + bf16 remainder ----
            hd1 = hpool.tile([128, 2, BST * ST], fp8, tag="hd0")
            hr1 = hpool.tile([128, BST * ST], bf16, tag="hr0")
            for m, (m0, mm) in enumerate(HCH):
                pss = []
                for g in range(gs):
                    p = mm_psum.tile([128, ST], f32, tag="mmp")
                    pss.append(p)
                for k, (get_rhs, kk) in enumerate(l1_ksrc):
                    lhs = W1ch[k][:, m0:m0 + mm]
                    for g in range(gs):
                        nc.tensor.matmul(pss[g][:mm, :], lhs, get_rhs(g),
                                         start=(k == 0),
                                         stop=(k == len(l1_ksrc) - 1))
                for g in range(gs):
                    dst = (hd1[:, m, g * ST:(g + 1) * ST] if m < 2
                           else hr1[:, g * ST:(g + 1) * ST])
                    nc.vector.tensor_scalar(
                        dst, pss[g][:mm, :], biases[0][m][:mm, :], 0.0,
                        op0=OP.add, op1=OP.max)

            # ---- L2 (fp8 DR) -> h2 fp8 dr-pair + bf16 remainder ----
            hd2 = hpool.tile([128, 2, BST * ST], fp8, tag="hd1")
            hr2 = hpool.tile([128, BST * ST], bf16, tag="hr1")
            for m, (m0, mm) in enumerate(HCH):
                pss = []
                for g in range(gs):
                    p = mm_psum.tile([128, ST], f32, tag="mmp")
                    pss.append(p)
                for g in range(gs):
                    nc.tensor.matmul(pss[g][:mm, :], w2d[:, :, m0:m0 + mm],
                                     hd1[:, :, g * ST:(g + 1) * ST],
                                     start=True, stop=False, perf_mode=DR)
                for g in range(gs):
                    nc.tensor.matmul(pss[g][:mm, :], w2r[:, m0:m0 + mm],
                                     hr1[:, g * ST:(g + 1) * ST],
                                     start=False, stop=True)
                for g in range(gs):
                    dst = (hd2[:, m, g * ST:(g + 1) * ST] if m < 2
                           else hr2[:, g * ST:(g + 1) * ST])
                    nc.scalar.activation(dst, pss[g][:mm, :], AF.Relu,
                                         bias=biases[1][m][:mm, :],
                                         scale=0.125)

            # ---- L3 (fp8 DR) -> h3 bf16 chunks ----
            cur = []
            for m, (m0, mm) in enumerate(HCH):
                pss = []
                for g in range(gs):
                    p = mm_psum.tile([128, ST], f32, tag="mmp")
                    pss.append(p)
                for g in range(gs):
                    nc.tensor.matmul(pss[g][:mm, :], w3d[:, :, m0:m0 + mm],
                                     hd2[:, :, g * ST:(g + 1) * ST],
                                     start=True, stop=False, perf_mode=DR)
                for g in range(gs):
                    nc.tensor.matmul(pss[g][:mm, :], w3r[:, m0:m0 + mm],
                                     hr2[:, g * ST:(g + 1) * ST],
                                     start=False, stop=True)
                ht = hpool.tile([128, BST * ST], bf16, tag=f"h2_{m}")
                for g in range(gs):
                    nc.scalar.activation(ht[:mm, g * ST:(g + 1) * ST],
                                         pss[g][:mm, :], AF.Relu,
                                         bias=biases[2][m][:mm, :],
                                         scale=0.125)
                cur.append(ht)
            return cur

        # ---- edge phase ----
        acc_state = {}

        def build_sts(blk):
            sts = []
            for i in range(16):
                cidx = blk * 16 + i
                st = spool.tile([128, NW], bf16, tag="st")
                nc.vector.tensor_scalar(
                    st[:, :], iota[:, :], dstloc[:, cidx:cidx + 1], None,
                    op0=OP.is_equal)
                sts.append(st)
            return sts

        def emit_scatter(blk, msgts, sts):
            for i, mt in enumerate(msgts):
                cidx = blk * 16 + i
                w = cidx // C
                cw = cidx % C
                if cw == 0:
                    at = acc_psum.tile([128, NW], f32, tag="acc")
                    acc_state["t"] = at
                nc.tensor.matmul(acc_state["t"][:, :], mt[:, :], sts[i][:, :],
                                 start=(cw == 0), stop=(cw == C - 1),
                                 skip_group_check=True)
                if cw == C - 1:
                    nc.scalar.activation(
                        aggrT[:, w * NW:(w + 1) * NW], acc_state["t"][:, :],
                        AF.Copy)

        prev_msgts = None
        prev_sts = None
        prev_blk = None
        for blk in range(NBLK):
            in_t = pending_in
            if blk + 1 < NBLK:
                pending_in = load_block(blk + 1)
            if prev_msgts is not None:
                prev_sts = build_sts(prev_blk)

            eksrc = [
                ((lambda g, t=t, kk=kk: t[:kk, g * ST:(g + 1) * ST]), kk)
                for t, (k0, kk) in zip(in_t, KIN)]

            h3 = mlp_front(eksrc, mW1, wdr["mW2"], wrem["mW2"],
                           wdr["mW3"], wrem["mW3"], mb, BST)

            # L4 weight-stationary: out [MSGD, ST] feature-major
            psl4 = []
            for g in range(BST):
                p4 = mm_psum.tile([128, ST], f32, tag="mmp")
                psl4.append(p4)
            for k, (k0, kk) in enumerate(HCH):
                for g in range(BST):
                    nc.tensor.matmul(psl4[g][:, :], mW4[k][:kk, :],
                                     h3[k][:kk, g * ST:(g + 1) * ST],
                                     start=(k == 0), stop=(k == 2))
            msgts = []
            for g in range(BST):
                mT = mtpool.tile([128, ST], bf16, tag=f"msgT{g}")
                nc.scalar.activation(mT[:, :], psl4[g][:, :], AF.Identity,
                                     bias=mb4c[:, :])
                mt4 = mtpool.tile([128, 4, 128], bf16, tag=f"msgt{g}")
                nc.sync.dma_start_transpose(mt4[:, :, :], mT[:, :])
                for e in range(4):
                    msgts.append(mt4[:, e, :])

            if prev_msgts is not None:
                emit_scatter(prev_blk, prev_msgts, prev_sts)
            prev_msgts, prev_blk = msgts, blk
        prev_sts = build_sts(prev_blk)
        emit_scatter(prev_blk, prev_msgts, prev_sts)

        # ---- node phase ----
        pool_acc = acc_psum.tile([128, NW], f32, tag="acc")
        for t0 in range(0, NT, BST):
            gs = min(BST, NT - t0)

            nksrc = [
                ((lambda g, s=xT, t0=t0:
                  s[:, (t0 + g) * ST:(t0 + g + 1) * ST]), NF),
                ((lambda g, s=aggrT, t0=t0:
                  s[:, (t0 + g) * ST:(t0 + g + 1) * ST]), MSGD)]

            h3n = mlp_front(nksrc, nW1, wdr["nW2"], wrem["nW2"],
                            wdr["nW3"], wrem["nW3"], nb, gs)

            for g in range(gs):
                for e in range(4):
                    tch = (t0 + g) * 4 + e
                    ps = mm_psum.tile([128, ST], f32, tag="mmp")
                    for k, (k0, kk) in enumerate(HCH):
                        nc.tensor.matmul(
                            ps[:, :NF],
                            h3n[k][:kk, g * ST + e * 128:g * ST + (e + 1) * 128],
                            nW4[k][:kk, :], start=(k == 0), stop=False)
                    nc.tensor.matmul(ps[:, :NF], ones1[:1, :], nb4r[:1, :],
                                     start=False, stop=True)
                    no = mpool.tile([128, NF], bf16, tag="no")
                    nc.scalar.activation(no[:, :], ps[:, :NF], AF.Copy)
                    nc.tensor.matmul(pool_acc[:G, :NF],
                                     pmat[:, tch * G:(tch + 1) * G], no[:, :],
                                     start=(tch == 0), stop=(tch == NCHK - 1),
                                     skip_group_check=True)

        pooled = apool.tile([G, NF], f32, tag="pooled")
        nc.vector.tensor_copy(pooled[:, :], pool_acc[:G, :NF])
        nc.sync.dma_start(d_out[:, :], pooled[:, :])

    nc.compile()
    return nc


def _prep_inputs(x, edge_index, edge_attr, batch, weights, C):
    """Host-side shard/gather/pad. Returns per-core in_maps."""
    E_pad = W * C * 128
    src = np.asarray(edge_index[0], np.int64)
    dst = np.asarray(edge_index[1], np.int64)

    order = np.argsort(dst, kind="stable")
    dsts = dst[order]
    srcs = src[order]

    xT = np.ascontiguousarray(np.asarray(x, np.float32).astype(BF16).T)
    eaT = np.ascontiguousarray(np.asarray(edge_attr, np.float32).astype(BF16).T)
    batch = np.asarray(batch, np.int64)

    bounds = np.searchsorted(dsts, np.arange(0, N_NODES + 1, NPC))

    def pad2(a, r, c):
        out = np.zeros((r, c), a.dtype)
        out[:a.shape[0], :a.shape[1]] = a
        return out

    FP8 = ml_dtypes.float8_e4m3fn
    wcommon = {}
    wcommon["mW1"] = pad2(weights["mW1"].astype(BF16), KINP, HIDP)
    wcommon["mW4"] = pad2(weights["mW4"].astype(BF16), HIDP, MSGD)
    wcommon["nW1"] = pad2(weights["nW1"].astype(BF16), NF + MSGD, HIDP)
    wcommon["nW4"] = pad2(weights["nW4"].astype(BF16), HIDP, NF)
    for nm in ("mW2", "mW3", "nW2", "nW3"):
        wp = pad2(weights[nm].astype(np.float32), HIDP, HIDP) * 8.0
        dr = wp[:256].reshape(2, 128, HIDP).transpose(1, 0, 2)
        wcommon[f"{nm}dr"] = np.ascontiguousarray(
            dr.reshape(128, 2 * HIDP).astype(FP8))
        wcommon[f"{nm}rem"] = np.ascontiguousarray(wp[256:HIDP].astype(BF16))
    for i in range(1, 4):
        wcommon[f"mb{i}"] = pad2(
            weights[f"mb{i}"].reshape(HID, 1).astype(np.float32), HIDP, 1)
        wcommon[f"nb{i}"] = pad2(
            weights[f"nb{i}"].reshape(HID, 1).astype(np.float32), HIDP, 1)
    wcommon["mb4c"] = np.ascontiguousarray(
        weights["mb4"].reshape(MSGD, 1).astype(np.float32))
    wcommon["nb4r"] = np.ascontiguousarray(
        weights["nb4"].reshape(1, NF).astype(BF16))

    garange = np.arange(G)
    in_maps = []
    for k in range(NCORES):
        sl = slice(int(bounds[k]), int(bounds[k + 1]))
        eidx = order[sl]
        dloc = dsts[sl] - k * NPC
        srck = srcs[sl]
        win = dloc // NW
        cnt = np.bincount(win, minlength=W)

        starts = np.repeat(np.arange(W) * C * 128, cnt)
        within = np.arange(len(dloc)) - np.repeat(np.cumsum(cnt) - cnt, cnt)
        pos = starts + within

        msg_inT = np.zeros((KINP, E_pad), BF16)
        msg_inT[0:NF, pos] = xT[:, k * NPC + dloc]
        msg_inT[NF:2 * NF, pos] = xT[:, srck]
        msg_inT[2 * NF:2 * NF + EF, pos] = eaT[:, eidx]

        dl = np.full(E_pad, -1.0, np.float32)
        dl[pos] = (dloc - win * NW).astype(np.float32)
        dstloc = np.ascontiguousarray(dl.reshape(E_pad // 128, 128).T)

        xTn = np.zeros((NF, NP2), BF16)
        xTn[:, :NPC] = xT[:, k * NPC:(k + 1) * NPC]

        bl = np.full(NP2, -1, np.int64)
        bl[:NPC] = batch[k * NPC:(k + 1) * NPC]
        P = (bl[:, None] == garange[None, :]).astype(BF16)
        pmat = np.ascontiguousarray(
            P.reshape(NCHK, 128, G).transpose(1, 0, 2).reshape(128, NCHK * G))

        in_map = dict(wcommon)
        in_map.update(msg_inT=msg_inT, dstloc=dstloc, xT=xTn, pmat=pmat)
        in_maps.append(in_map)
    return in_maps


def kernel(**inputs):
    global LAST_EXEC_NS
    from concourse.bass_utils import run_bass_kernel_spmd

    x = np.asarray(inputs["x"], np.float32)
    edge_index = np.asarray(inputs["edge_index"])
    edge_attr = np.asarray(inputs["edge_attr"], np.float32)
    batch = np.asarray(inputs["batch"])

    # chunk count per window from the actual data (uniform across cores)
    dst = np.asarray(edge_index[1], np.int64)
    dloc_all = dst % NPC
    core_all = dst // NPC
    win_all = dloc_all // NW
    cnt = np.bincount(core_all * W + win_all, minlength=NCORES * W)
    C = int(np.ceil(cnt.max() / 128.0))
    C = max(C, 8)
    while (W * C) % 16 != 0:
        C += 1

    key = C
    if key not in _BUILD_CACHE:
        _BUILD_CACHE[key] = _build_nc(C)
    nc = _BUILD_CACHE[key]

    in_maps = _prep_inputs(x, edge_index, edge_attr, batch, inputs, C)

    res = run_bass_kernel_spmd(nc, in_maps, list(range(NCORES)), trace=TRACE)
    LAST_EXEC_NS = res.exec_time_ns

    total = np.zeros((G, NF), np.float64)
    for r in res.results:
        total += np.asarray(r["partial"], np.float64)

    counts = np.bincount(np.asarray(batch, np.int64), minlength=G)
    pooled = (total / np.maximum(counts, 1)[:, None]).astype(np.float32)
    out = pooled @ np.asarray(inputs["linW"], np.float32) + np.asarray(
        inputs["linb"], np.float32)
    return out.astype(np.float32)
